# revision 1
# baseline (speedup 1.0000x reference)
"""Trainium2 Bass kernel for nn_Block (attention + soft top-2 MoE), 8-core SPMD.

v2: - Router logits via exact free-rider path: the AV matmul carries 19 extra
      columns per head (VP_hi/lo = V @ (Wproj@W~r) head-block, f22/bf16 split)
      sharing the truncated-ex weights with the ones-denominator, so
      num/den cancellation keeps top-2 margins exact to ~3e-6 while the whole
      pipeline stays 1-pass f32r.  logits = (x@W~r + sum_h num_h/den_h
      - mu*s) * rstd, with x@W~r - s (x) mu^T as one small f32 4-pass matmul.
    - fp8e4 DoubleRow FFN (weights pre-scaled x32 on host).
    - bf16 on the wire for V', h2, w, and the ReduceScatter.
    - Attention restructured: head-pairs x chunks, one exp per [128,1024].
"""

import sys

if "/opt/trn_rl_repo" not in sys.path:
    sys.path.insert(0, "/opt/trn_rl_repo")

import numpy as np
import ml_dtypes

import concourse.bass as bass
import concourse.mybir as mybir
import concourse.tile as tile
from concourse import bacc
from concourse.masks import make_identity

F32 = mybir.dt.float32
F32R = mybir.dt.float32r
BF16 = mybir.dt.bfloat16
FP8 = mybir.dt.float8e4
I32 = mybir.dt.int32
AF = mybir.ActivationFunctionType
ALU = mybir.AluOpType
DR = mybir.MatmulPerfMode.DoubleRow

B, T, D = 2, 2048, 1024
H, HD = 16, 64
E = 8
NC = 8
N = B * T
OWN = N // NC              # 512
NCH = 16
DFF = 4 * D
CAP = 1024
VA = 74                    # 64 v + 1 ones(den) + 9 VP
EPS = 1e-5
RG = [list(range(NC))]
WS = 32.0                  # fp8 weight pre-scale


def r32(x):
    return x.bitcast(F32R)


def f32(x):
    return x.bitcast(F32)


def build_nc():
    nc = bacc.Bacc("TRN2", target_bir_lowering=False, debug=False,
                   num_devices=NC)

    x_own = nc.dram_tensor("x_own", [OWN, D], F32, kind="ExternalInput")
    wqkv = nc.dram_tensor("wqkv", [D, 3 * D], F32R, kind="ExternalInput")
    wvp = nc.dram_tensor("wvp", [D, H * 9], F32, kind="ExternalInput")
    wproj = nc.dram_tensor("wproj", [D, D], F32R, kind="ExternalInput")
    wrouter = nc.dram_tensor("wrouter", [D, E + 1], F32, kind="ExternalInput")
    srow = nc.dram_tensor("srow", [1, E + 1], F32, kind="ExternalInput")
    wfc8 = nc.dram_tensor("wfc8", [D, DFF], FP8, kind="ExternalInput")
    wpj8 = nc.dram_tensor("wpj8", [DFF, D], FP8, kind="ExternalInput")
    ln2bc = nc.dram_tensor("ln2bc", [128, D], F32, kind="ExternalInput")
    tri = nc.dram_tensor("tri", [128, 128], F32R, kind="ExternalInput")
    dmask = nc.dram_tensor("dmask", [128, 4 * OWN], BF16, kind="ExternalInput")
    tid = nc.dram_tensor("tid", [128, 32], I32, kind="ExternalInput")
    ksel = nc.dram_tensor("ksel", [128, NCH], I32, kind="ExternalInput")
    emask = nc.dram_tensor("emask", [128, 16], F32, kind="ExternalInput")
    sgrid = nc.dram_tensor("sgrid", [128, 8], F32, kind="ExternalInput")
    iota32 = nc.dram_tensor("iota32", [128, 32], F32, kind="ExternalInput")
    out = nc.dram_tensor("out", [OWN, D], F32, kind="ExternalOutput")

    agin_k = nc.dram_tensor("agin_k", [OWN, D], F32)
    agout_k = nc.dram_tensor("agout_k", [N + 64, D], F32, addr_space="Shared")
    agin_v = nc.dram_tensor("agin_v", [OWN, H * VA], F32R)
    agout_v = nc.dram_tensor("agout_v", [N + 64, H * VA], F32R,
                             addr_space="Shared")
    agin_h2 = nc.dram_tensor("agin_h2", [OWN, D], BF16)
    agout_h2 = nc.dram_tensor("agout_h2", [N + 64, D], BF16,
                              addr_space="Shared")
    agin_w = nc.dram_tensor("agin_w", [OWN, 16], BF16)
    agout_w = nc.dram_tensor("agout_w", [N + 64, 16], BF16,
                             addr_space="Shared")
    idx_d = nc.dram_tensor("idx_d", [1152, 1], I32)
    idxL = [nc.dram_tensor(f"idxL{f}", [136, 1], I32) for f in range(32)]
    idxAll = nc.dram_tensor("idxAll", [6272, 1], I32)
    ffnout = nc.dram_tensor("ffnout", [1152, D], BF16)
    yt2_d = nc.dram_tensor("yt2_d", [128, 8 * OWN], F32R)
    xmid_d = nc.dram_tensor("xmid_d", [OWN, D], F32)
    rsin = nc.dram_tensor("rsin", [N + 64, D], BF16)
    rsout = nc.dram_tensor("rsout", [OWN, D], BF16)

    with tile.TileContext(nc) as tc:
        build_body(nc, tc, locals())
    nc.compile()
    return nc


def build_body(nc, tc, t):
    x_own, wqkv, wvp, wproj, wrouter, srow = (
        t["x_own"], t["wqkv"], t["wvp"], t["wproj"], t["wrouter"], t["srow"])
    wfc8, wpj8, ln2bc, tri, dmask, tid, ksel, emask = (
        t["wfc8"], t["wpj8"], t["ln2bc"], t["tri"], t["dmask"], t["tid"],
        t["ksel"], t["emask"])
    sgrid, iota32 = t["sgrid"], t["iota32"]
    agin_k, agout_k, agin_v, agout_v = (
        t["agin_k"], t["agout_k"], t["agin_v"], t["agout_v"])
    agin_h2, agout_h2, agin_w, agout_w = (
        t["agin_h2"], t["agout_h2"], t["agin_w"], t["agout_w"])
    idx_d, ffnout, yt2_d, xmid_d, rsin, rsout, out = (
        t["idx_d"], t["ffnout"], t["yt2_d"], t["xmid_d"], t["rsin"],
        t["rsout"], t["out"])
    idxL, idxAll = t["idxL"], t["idxAll"]

    ident_p = tc.alloc_tile_pool(name="ident", bufs=1)
    ident = ident_p.tile([128, 128], F32)
    make_identity(nc, ident[:])
    identb = ident_p.tile([128, 128], BF16)
    nc.vector.tensor_copy(identb[:], ident[:])

    cst_p = tc.alloc_tile_pool(name="cst", bufs=1)
    ksel_sb = cst_p.tile([128, NCH], I32)
    nc.sync.dma_start(ksel_sb[:], ksel[:, :])
    emask_sb = cst_p.tile([128, 16], F32)
    nc.sync.dma_start(emask_sb[:], emask[:, :])
    ones_cf = cst_p.tile([128, 1], F32)
    nc.vector.memset(ones_cf[:], 1.0)
    ones_c = cst_p.tile([128, 1], F32R)
    nc.vector.tensor_copy(ones_c[:], ones_cf[:])
    ones_rf = cst_p.tile([1, 128], F32)
    nc.vector.memset(ones_rf[:], 1.0)
    ones_r = cst_p.tile([1, 128], F32R)
    nc.vector.tensor_copy(ones_r[:], ones_rf[:])
    eps_sb = cst_p.tile([128, 1], F32)
    nc.vector.memset(eps_sb[:], EPS)
    nc.eps_sb = eps_sb
    w8 = cst_p.tile([128, 4], F32)              # identity-expert weight
    srow_sb = cst_p.tile([1, E + 1], F32)
    nc.sync.dma_start(srow_sb[:], srow[:, :])

    # persistent SBUF across phases 1-3
    per_p = tc.alloc_tile_pool(name="per", bufs=1)
    qT = per_p.tile([128, 8 * OWN], F32R)        # [d, q] scaled 1/8
    ndg = [per_p.tile([128, OWN], F32, name=f"ndg{i}") for i in range(4)]
    ndT = per_p.tile([128, 4 * OWN], F32)        # transposed nd per tt
    xr0 = per_p.tile([E + 1, OWN], F32)          # wr^T @ x^T, [9, tok]
    xrT = per_p.tile([E + 1, OWN], F32)          # + s (x) (-mu), [9, tok]
    nmean4 = per_p.tile([128, 4], F32)           # -mu per tt
    rstd4 = per_p.tile([128, 4], F32)

    # ---------------- Phase 1: LN1 + QKV + VP + xT + AllGather ----------
    with tc.tile_pool(name="p1", bufs=2) as p1, \
         tc.tile_pool(name="p1w", bufs=2) as p1w:
        p1ps_cm = tc.tile_pool(name="p1psA", bufs=2, space="PSUM")
        p1ps = p1ps_cm.__enter__()
        xlnT = p1.tile([128, 8 * OWN], F32R, tag="xlnT")
        xT = p1.tile([128, 8 * OWN], F32, tag="xT")
        wvp_sb = p1.tile([128, 8 * H * 9], F32, tag="wvp")
        nc.sync.dma_start(
            wvp_sb[:].rearrange("p (dt m) -> p dt m", m=H * 9),
            wvp[:, :].rearrange("(dt p) m -> p dt m", p=128))
        for tt in range(4):
            xs = p1.tile([128, D], F32, tag="xs")
            nc.sync.dma_start(xs[:], x_own[tt * 128:(tt + 1) * 128, :])
            for dt in range(8):
                pst = p1ps.tile([128, 128], F32, tag="tpx")
                nc.tensor.transpose(pst[:], xs[:, dt * 128:(dt + 1) * 128],
                                    ident[:])
                nc.vector.tensor_copy(
                    xT[:, dt * OWN + tt * 128: dt * OWN + (tt + 1) * 128],
                    pst[:])
            xln, nmean, rstd = _layernorm(nc, p1, xs, D)
            for dt in range(8):
                pst = p1ps.tile([128, 128], F32, tag="tp")
                nc.tensor.transpose(pst[:], xln[:, dt * 128:(dt + 1) * 128],
                                    ident[:])
                nc.vector.tensor_copy(
                    xlnT[:, dt * OWN + tt * 128: dt * OWN + (tt + 1) * 128],
                    pst[:])
        # q: scaled 1/8
        for mc in range(8):
            wp = p1w.tile([128, 1024], F32R, tag="wq")
            nc.sync.dma_start(
                wp[:].rearrange("p (dt m) -> p dt m", m=128),
                wqkv[:, mc * 128:(mc + 1) * 128].rearrange(
                    "(dt p) m -> p dt m", p=128))
            psq = p1ps.tile([128, OWN], F32, tag="psq")
            for dt in range(8):
                nc.tensor.matmul(
                    psq[:], r32(wp[:, dt * 128:(dt + 1) * 128]),
                    r32(xlnT[:, dt * OWN:(dt + 1) * OWN]),
                    start=(dt == 0), stop=(dt == 7))
            nc.vector.tensor_scalar_mul(qT[:, mc * OWN:(mc + 1) * OWN],
                                        psq[:], 0.125)
        # VP = xln @ Wvp (f32 4-pass), staged to SBUF for the v-assembly
        vpsb = p1.tile([128, 4 * H * 9], F32, tag="vpsb")
        for tt in range(4):
            pvp = p1ps.tile([128, H * 9], F32, tag="pvp")
            for dt2 in range(8):
                nc.tensor.matmul(
                    pvp[:],
                    f32(xlnT[:, dt2 * OWN + tt * 128:
                             dt2 * OWN + (tt + 1) * 128]),
                    wvp_sb[:, dt2 * H * 9:(dt2 + 1) * H * 9],
                    start=(dt2 == 0), stop=(dt2 == 7))
            nc.vector.tensor_copy(
                vpsb[:, tt * H * 9:(tt + 1) * H * 9], pvp[:])
        p1ps_cm.__exit__(None, None, None)
        p1ps_cm2 = tc.tile_pool(name="p1psB", bufs=1, space="PSUM")
        p1ps = p1ps_cm2.__enter__()
        # k and v token-major; VP via f32 4-pass
        for sec in range(2):  # 0 = k, 1 = v
            pk0 = p1ps.tile([128, D], F32, tag="pkv0")
            pk1 = p1ps.tile([128, D], F32, tag="pkv1")
            pk2 = p1ps.tile([128, D], F32, tag="pkv2")
            pk3 = p1ps.tile([128, D], F32, tag="pkv3")
            pks = [pk0, pk1, pk2, pk3]
            for dt in range(8):
                wp = p1w.tile([128, 1024], F32R, tag="wkv")
                nc.sync.dma_start(
                    wp[:], wqkv[dt * 128:(dt + 1) * 128,
                                (1 + sec) * D:(2 + sec) * D])
                for tt in range(4):
                    pk = pks[tt]
                    for half in range(2):
                        nc.tensor.matmul(
                            pk[:, half * 512:(half + 1) * 512],
                            r32(xlnT[:, dt * OWN + tt * 128:
                                     dt * OWN + (tt + 1) * 128]),
                            r32(wp[:, half * 512:(half + 1) * 512]),
                            start=(dt == 0), stop=(dt == 7))
                    if dt == 7:
                        if sec == 0:
                            ks = p1.tile([128, D], F32, tag="ko")
                            nc.vector.tensor_copy(ks[:], pk[:])
                            nc.sync.dma_start(
                                agin_k[tt * 128:(tt + 1) * 128, :], ks[:])
                        else:
                            pvp = vpsb[:, tt * H * 9:(tt + 1) * H * 9]
                            vs = p1.tile([128, H * VA], F32, tag="vo")
                            vv = vs[:].rearrange("p (h c) -> p h c", c=VA)
                            nc.vector.memset(vv[:, :, 64:65], 1.0)
                            nc.vector.tensor_copy(
                                vv[:, :, 0:64],
                                pk[:].rearrange("p (h c) -> p h c", c=64))
                            nc.vector.tensor_copy(
                                vv[:, :, 65:74],
                                pvp.rearrange("p (h c) -> p h c", c=9))
                            nc.gpsimd.dma_start(
                                agin_v[tt * 128:(tt + 1) * 128, :], vs[:])
        p1ps_cm2.__exit__(None, None, None)
        # xr0 = wr^T @ x^T  (f32 4-pass, [9, OWN])
        p1ps_cm3 = tc.tile_pool(name="p1psC", bufs=1, space="PSUM")
        p1ps3 = p1ps_cm3.__enter__()
        wr_sb = p1.tile([128, 8 * (E + 1)], F32, tag="wr")
        nc.sync.dma_start(
            wr_sb[:].rearrange("p (dt m) -> p dt m", m=E + 1),
            wrouter[:, :].rearrange("(dt p) m -> p dt m", p=128))
        pxr = p1ps3.tile([E + 1, OWN], F32, tag="pxr")
        for dt in range(8):
            nc.tensor.matmul(
                pxr[:], wr_sb[:, dt * (E + 1):(dt + 1) * (E + 1)],
                xT[:, dt * OWN:(dt + 1) * OWN],
                start=(dt == 0), stop=(dt == 7))
        nc.vector.tensor_copy(xr0[:], pxr[:])
        p1ps_cm3.__exit__(None, None, None)
        nc.gpsimd.collective_compute(
            "AllGather", ALU.bypass, replica_groups=RG,
            ins=[agin_k[:, :].opt()], outs=[agout_k[0:N, :].opt()])
        nc.gpsimd.collective_compute(
            "AllGather", ALU.bypass, replica_groups=RG,
            ins=[agin_v[:, :].opt()], outs=[agout_v[0:N, :].opt()])
        zr = p1.tile([64, D], F32, tag="zr")
        nc.vector.memset(zr[:], 0.0)
        nc.sync.dma_start(agout_k[N:N + 64, :], zr[:, 0:D])
        zrb = p1.tile([64, H * VA], F32, tag="zrb")
        nc.vector.memset(zrb[:], 0.0)
        nc.gpsimd.dma_start(agout_v[N:N + 64, :], zrb[:])
        zrow = p1.tile([128, D], BF16, tag="zrow")
        nc.vector.memset(zrow[:], 0.0)
        nc.sync.dma_start(agout_h2[N:N + 64, :], zrow[0:64, :])
        nc.sync.dma_start(agout_w[N:N + 64, :], zrow[0:64, 0:16])
        nc.sync.dma_start(ffnout[CAP:CAP + 128, :], zrow[:])

    # ---------------- Phase 2: attention ----------------
    with tc.tile_pool(name="p2", bufs=1) as p2, \
         tc.tile_pool(name="p2s", bufs=2) as p2s:
        dm_sb = p2.tile([128, 4 * OWN], BF16, tag="dm")
        nc.sync.dma_start(dm_sb[:], dmask[:, :])
        kT = p2.tile([128, 8 * 2048], F32R, tag="kT")
        vall = p2.tile([128, NCH * H * VA], F32R, tag="vall")
        p2ps_cm = tc.tile_pool(name="p2psT", bufs=2, space="PSUM")
        p2ps = p2ps_cm.__enter__()
        p2k_cm = tc.tile_pool(name="p2k", bufs=2)
        p2k = p2k_cm.__enter__()
        for ch in range(NCH):
            kch = p2k.tile([128, D], F32, tag="kch")
            nc.gpsimd.indirect_dma_start(
                out=kch[:], out_offset=None, in_=agout_k[:, :],
                in_offset=bass.IndirectOffsetOnAxis(
                    ap=ksel_sb[:, ch:ch + 1], axis=0))
            for dt in range(8):
                pst = p2ps.tile([128, 128], F32, tag="tp2")
                nc.tensor.transpose(pst[:], kch[:, dt * 128:(dt + 1) * 128],
                                    ident[:])
                nc.vector.tensor_copy(
                    kT[:, dt * 2048 + ch * 128:dt * 2048 + (ch + 1) * 128],
                    pst[:])
        for ch in range(NCH):
            nc.gpsimd.indirect_dma_start(
                out=vall[:, ch * H * VA:(ch + 1) * H * VA], out_offset=None,
                in_=agout_v[:, :],
                in_offset=bass.IndirectOffsetOnAxis(
                    ap=ksel_sb[:, ch:ch + 1], axis=0))
        p2k_cm.__exit__(None, None, None)
        p2ps_cm.__exit__(None, None, None)
        p2ps_cm2 = tc.tile_pool(name="p2psB", bufs=2, space="PSUM")
        p2ps = p2ps_cm2.__enter__()
        p2psc_cm = tc.tile_pool(name="p2psC", bufs=2, space="PSUM")
        p2psc = p2psc_cm.__enter__()
        p2psy_cm = tc.tile_pool(name="p2psY", bufs=1, space="PSUM")
        p2psy = p2psy_cm.__enter__()
        for hp in range(8):                       # head pairs
            ypss = []
            for hh in range(2):
                yps = p2psy.tile([VA, OWN], F32, tag=f"yps{hh}")
                ypss.append(yps)
            for ch in range(NCH):
                psc = p2ps.tile([128, 2 * OWN], F32, tag="psc")
                for hh in range(2):
                    h = hp * 2 + hh
                    dt, ph = h // 2, (h % 2) * 64
                    nc.tensor.matmul(
                        psc[:, hh * OWN:(hh + 1) * OWN],
                        r32(kT[ph:ph + 64, dt * 2048 + ch * 128:
                               dt * 2048 + (ch + 1) * 128]),
                        r32(qT[ph:ph + 64, dt * OWN:(dt + 1) * OWN]),
                        start=True, stop=True)
                ex = p2s.tile([128, 2 * OWN], F32R, tag="ex")
                nc.scalar.activation(ex[:], psc[:], AF.Exp)
                if ch < 4:
                    nc.vector.tensor_tensor(
                        out=ex[:].rearrange("p (g m) -> p g m", g=2),
                        in0=ex[:].rearrange("p (g m) -> p g m", g=2),
                        in1=dm_sb[:, ch * OWN:(ch + 1) * OWN].rearrange(
                            "p (g m) -> p g m", g=1).to_broadcast(
                            [128, 2, OWN]),
                        op=ALU.mult)
                for hh in range(2):
                    h = hp * 2 + hh
                    nc.tensor.matmul(
                        ypss[hh][:],
                        r32(vall[:, ch * H * VA + h * VA:
                                 ch * H * VA + (h + 1) * VA]),
                        r32(ex[:, hh * OWN:(hh + 1) * OWN]),
                        start=(ch == 0), stop=(ch == NCH - 1))
            for hh in range(2):
                h = hp * 2 + hh
                dt, ph = h // 2, (h % 2) * 64
                yps = ypss[hh]
                rin = p2s.tile([1, OWN], F32R, tag="rin")
                with nc.allow_low_precision(reason="f32r rhs for bcast mm"):
                    nc.vector.reciprocal(rin[:], yps[64:65, :])
                pbc = p2psc.tile([64, OWN], F32, tag="pbc")
                nc.tensor.matmul(pbc[:], r32(ones_r[:, 0:64]), r32(rin[:]),
                                 start=True, stop=True)
                pbs = p2s.tile([64, OWN], F32, tag="pbs")
                nc.vector.tensor_copy(pbs[:], pbc[:])
                nc.vector.tensor_tensor(
                    out=pbs[:], in0=yps[0:64, :], in1=pbs[:], op=ALU.mult)
                nc.sync.dma_start(
                    yt2_d[ph:ph + 64, dt * OWN:(dt + 1) * OWN],
                    r32(pbs[:]))
                # stack den+num rows for the router path (32-part aligned)
                grp, slot = h // 4, h % 4
                nc.vector.tensor_copy(
                    ndg[grp][32 * slot:32 * slot + 10, :], yps[64:74, :])
        p2psy_cm.__exit__(None, None, None)
        p2psc_cm.__exit__(None, None, None)
        p2ps_cm2.__exit__(None, None, None)
        # transpose nd stacks to token-major [128, 4*128] per tt
        p2psd_cm = tc.tile_pool(name="p2psD", bufs=2, space="PSUM")
        p2psd = p2psd_cm.__enter__()
        for grp in range(4):
            for tt in range(4):
                pst = p2psd.tile([128, 128], F32, tag="tpn")
                nc.tensor.transpose(
                    pst[:], ndg[grp][:, tt * 128:(tt + 1) * 128], ident[:])
                nc.vector.tensor_copy(
                    ndT[:, tt * 512 + grp * 128:tt * 512 + (grp + 1) * 128],
                    pst[:])
        p2psd_cm.__exit__(None, None, None)

    # ------------- Phase 3: proj + residual + LN2 + logits + router ------
    with tc.tile_pool(name="p3", bufs=2) as p3, \
         tc.tile_pool(name="p3w", bufs=2) as p3w:
        p3ps_cm = tc.tile_pool(name="p3psA", bufs=2, space="PSUM")
        p3ps = p3ps_cm.__enter__()
        muT = p3.tile([1, OWN], F32, tag="muT")
        yT2 = p3.tile([128, 8 * OWN], F32R, tag="yT2")
        nc.sync.dma_start(yT2[:], yt2_d[:, :])
        for tt in range(4):
            pp = p3ps.tile([128, D], F32, tag="pp")
            for dt in range(8):
                wp = p3w.tile([128, D], F32R, tag="wpj3")
                nc.sync.dma_start(wp[:], wproj[dt * 128:(dt + 1) * 128, :])
                for half in range(2):
                    nc.tensor.matmul(
                        pp[:, half * 512:(half + 1) * 512],
                        r32(yT2[:, dt * OWN + tt * 128:
                                dt * OWN + (tt + 1) * 128]),
                        r32(wp[:, half * 512:(half + 1) * 512]),
                        start=(dt == 0), stop=(dt == 7))
            xot = p3.tile([128, D], F32, tag="xot")
            nc.sync.dma_start(xot[:], x_own[tt * 128:(tt + 1) * 128, :])
            xmt = p3.tile([128, D], F32, tag="xmt")
            nc.vector.tensor_add(xmt[:], xot[:], pp[:])
            nc.sync.dma_start(xmid_d[tt * 128:(tt + 1) * 128, :], xmt[:])
            h2t, nmean, rstd = _layernorm(nc, p3, xmt, D)
            nc.vector.tensor_copy(nmean4[:, tt:tt + 1], nmean[:])
            nc.vector.tensor_copy(rstd4[:, tt:tt + 1], rstd[:])
            h2b = p3.tile([128, D], BF16, tag="h2b")
            nc.vector.tensor_copy(h2b[:], h2t[:])
            nc.sync.dma_start(agin_h2[tt * 128:(tt + 1) * 128, :], h2b[:])
            # -mu^T strip via transpose
            psm = p3ps.tile([1, 128], F32, tag="psm")
            nc.tensor.transpose(psm[:], nmean4[:, tt:tt + 1], ident[:])
            nc.vector.tensor_copy(muT[:, tt * 128:(tt + 1) * 128], psm[:])
        # xrT = xr0 + s^T (x) (-mu^T)
        pxr2 = p3ps.tile([E + 1, OWN], F32, tag="pxr2")
        nc.tensor.matmul(pxr2[:], srow_sb[:], muT[:], start=True, stop=True)
        nc.vector.tensor_add(xrT[:], xr0[:], pxr2[:])
        p3ps_cm.__exit__(None, None, None)
        p3ps_cm2 = tc.tile_pool(name="p3psB", bufs=2, space="PSUM")
        p3ps = p3ps_cm2.__enter__()
        for tt in range(4):
            pxt = p3ps.tile([128, E + 1], F32, tag="pxt")
            nc.tensor.transpose(pxt[:], xrT[:, tt * 128:(tt + 1) * 128],
                                ident[0:E + 1, 0:E + 1])
            # per-head num*recip(den), summed over heads
            lt = p3.tile([128, E + 1], F32, tag="lt")
            nc.vector.tensor_copy(lt[:], pxt[:])
            for grp in range(4):
                ndt = ndT[:, tt * 512 + grp * 128:tt * 512 + (grp + 1) * 128]
                nd3 = ndt.rearrange("p (h c) -> p h c", c=32)
                rec = p3.tile([128, 4], F32, tag="rec")
                nc.vector.reciprocal(
                    rec[:], nd3[:, :, 0:1].rearrange("p h c -> p (h c)"))
                sc8 = p3.tile([128, 36], F32, tag="sc8")
                nc.vector.tensor_tensor(
                    out=sc8[:].rearrange("p (j h) -> p h j", h=4),
                    in0=nd3[:, :, 1:10],
                    in1=rec[:].rearrange("p (h c) -> p h c", c=1)
                    .to_broadcast([128, 4, 9]),
                    op=ALU.mult)
                ssum = p3.tile([128, E + 1], F32, tag="ssum")
                nc.vector.reduce_sum(
                    ssum[:], sc8[:].rearrange("p (j h) -> p j h", h=4),
                    axis=mybir.AxisListType.X)
                nc.vector.tensor_add(lt[:], lt[:], ssum[:])
            nc.vector.tensor_scalar_mul(lt[:], lt[:], rstd4[:, tt:tt + 1])
            # softmax + top-2 weights on [128, 9]
            rmax = p3.tile([128, 1], F32, tag="rmax")
            nc.vector.reduce_max(rmax[:], lt[:], axis=mybir.AxisListType.X)
            nrm = p3.tile([128, 1], F32, tag="nrm")
            nc.vector.tensor_scalar_mul(nrm[:], rmax[:], -1.0)
            prob = p3.tile([128, E + 1], F32, tag="prob")
            sume = p3.tile([128, 1], F32, tag="sume")
            nc.scalar.activation(prob[:], lt[:], AF.Exp, bias=nrm[:],
                                 accum_out=sume[:])
            rinv = p3.tile([128, 1], F32, tag="rinv")
            nc.vector.reciprocal(rinv[:], sume[:])
            nc.scalar.activation(prob[:], prob[:], AF.Copy, scale=rinv[:])
            m1 = p3.tile([128, 1], F32, tag="m1")
            nc.vector.reduce_max(m1[:], prob[:], axis=mybir.AxisListType.X)
            eq = p3.tile([128, E + 1], F32, tag="eq")
            nc.vector.tensor_tensor(
                out=eq[:], in0=prob[:], in1=m1[:].to_broadcast([128, E + 1]),
                op=ALU.is_equal)
            pm = p3.tile([128, E + 1], F32, tag="pm")
            nc.vector.tensor_scalar_mul(pm[:], eq[:], -2.0)
            nc.vector.tensor_add(pm[:], pm[:], prob[:])
            m2 = p3.tile([128, 1], F32, tag="m2")
            nc.vector.reduce_max(m2[:], pm[:], axis=mybir.AxisListType.X)
            ge = p3.tile([128, E + 1], F32, tag="ge")
            nc.vector.tensor_tensor(
                out=ge[:], in0=prob[:], in1=m2[:].to_broadcast([128, E + 1]),
                op=ALU.is_ge)
            w16 = p3.tile([128, 16], F32, tag="w16")
            nc.vector.memset(w16[:], 0.0)
            nc.vector.tensor_mul(w16[:, 0:E + 1], prob[:], ge[:])
            nc.vector.tensor_copy(w8[:, tt:tt + 1], w16[:, E:E + 1])
            w16b = p3.tile([128, 16], BF16, tag="w16b")
            nc.vector.tensor_copy(w16b[:], w16[:])
            nc.sync.dma_start(agin_w[tt * 128:(tt + 1) * 128, :], w16b[:])
        p3ps_cm2.__exit__(None, None, None)
    nc.gpsimd.collective_compute(
        "AllGather", ALU.bypass, replica_groups=RG,
        ins=[agin_w[:, :].opt()], outs=[agout_w[0:N, :].opt()])

    # ---------------- Phase 4: routing compaction ----------------
    cmp_p = tc.alloc_tile_pool(name="cmp", bufs=1)
    slot_i = cmp_p.tile([128, 32], I32)
    idx_sb = cmp_p.tile([128, 8], I32)
    wslot = cmp_p.tile([128, 8], F32)
    with tc.tile_pool(name="p4", bufs=1) as p4, \
         tc.tile_pool(name="p4ps", bufs=1, space="PSUM") as p4ps:
        wfull = p4.tile([128, 32, 16], BF16, tag="wfull")
        nc.sync.dma_start(
            wfull[:], agout_w[0:N, :].rearrange("(f p) c -> p f c", p=128))
        wsel = p4.tile([128, 32, 16], F32, tag="wsel")
        nc.vector.tensor_tensor(
            out=wsel[:], in0=wfull[:],
            in1=emask_sb[:].rearrange("p (o c) -> p o c", o=1).to_broadcast(
                [128, 32, 16]),
            op=ALU.mult)
        wcol = p4.tile([128, 32], F32, tag="wcol")
        nc.vector.reduce_sum(wcol[:], wsel[:], axis=mybir.AxisListType.X)
        g01 = p4.tile([128, 32], F32R, tag="g01")
        nc.vector.tensor_scalar(out=g01[:], in0=wcol[:], scalar1=0.0,
                                scalar2=None, op0=ALU.is_gt)
        pcs = p4ps.tile([1, 32], F32, tag="pcs")
        nc.tensor.matmul(pcs[:], r32(ones_c[:]), r32(g01[:]), start=True,
                         stop=True)
        csum = p4.tile([1, 32], F32, tag="csum")
        nc.vector.tensor_copy(csum[:], pcs[:])
        pfx0 = p4.tile([1, 32], F32, tag="pfx0")
        pfx1 = p4.tile([1, 32], F32, tag="pfx1")
        pfx = [pfx0, pfx1]
        cur = csum
        for i, sh in enumerate([1, 2, 4, 8, 16]):
            nxt = pfx[i % 2]
            nc.vector.tensor_add(nxt[:, sh:32], cur[:, sh:32],
                                 cur[:, 0:32 - sh])
            nc.vector.tensor_copy(nxt[:, 0:sh], cur[:, 0:sh])
            cur = nxt
        exclf = p4.tile([1, 32], F32, tag="exclf")
        nc.vector.memset(exclf[:, 0:1], 0.0)
        nc.vector.tensor_copy(exclf[:, 1:32], cur[:, 0:31])
        excl = p4.tile([1, 32], F32R, tag="excl")
        nc.vector.tensor_copy(excl[:], exclf[:])
        tri_sb = p4.tile([128, 128], F32R, tag="tri")
        nc.sync.dma_start(tri_sb[:], tri[:, :])
        # local (within-chunk) rank, and broadcast excl/inc tables
        psl1 = p4ps.tile([128, 32], F32, tag="psl1")
        nc.tensor.matmul(psl1[:], r32(tri_sb[:]), r32(g01[:]), start=True,
                         stop=True)
        slf1 = p4.tile([128, 32], F32, tag="slf1")
        nc.vector.tensor_copy(slf1[:], psl1[:])
        pexb = p4ps.tile([128, 32], F32, tag="pexb")
        nc.tensor.matmul(pexb[:], r32(ones_r[:]), r32(excl[:]), start=True,
                         stop=True)
        exclb = p4.tile([128, 32], F32, tag="exclb")
        nc.vector.tensor_copy(exclb[:], pexb[:])
        csumr = p4.tile([1, 32], F32R, tag="csumr")
        nc.vector.tensor_copy(csumr[:], csum[:])
        pinb = p4ps.tile([128, 32], F32, tag="pinb")
        nc.tensor.matmul(pinb[:], r32(ones_r[:]), r32(csumr[:]), start=True,
                         stop=True)
        incb = p4.tile([128, 32], F32, tag="incb")
        nc.vector.tensor_add(incb[:], exclb[:], pinb[:])
        # global slot (for the phase-6 return scatter-gather)
        slf = p4.tile([128, 32], F32, tag="slf")
        nc.vector.tensor_add(slf[:], slf1[:], exclb[:])
        nc.vector.tensor_scalar_add(slf[:], slf[:], -float(CAP))
        nc.vector.tensor_mul(slf[:], slf[:], g01[:])
        nc.vector.tensor_scalar(out=slf[:], in0=slf[:], scalar1=float(CAP),
                                scalar2=float(CAP), op0=ALU.add, op1=ALU.min)
        nc.vector.tensor_copy(slot_i[:], slf[:])
        # level-1: independent per-chunk scatters at local ranks
        slfL = p4.tile([128, 32], F32, tag="slfL")
        nc.vector.tensor_scalar_add(slfL[:], slf1[:], -128.0)
        nc.vector.tensor_mul(slfL[:], slfL[:], g01[:])
        nc.vector.tensor_scalar_add(slfL[:], slfL[:], 128.0)
        slotL_i = p4.tile([128, 32], I32, tag="slotL_i")
        nc.vector.tensor_copy(slotL_i[:], slfL[:])
        tid_sb = p4.tile([128, 32], I32, tag="tid")
        nc.sync.dma_start(tid_sb[:], tid[:, :])
        for f in range(32):
            nc.gpsimd.indirect_dma_start(
                out=idxL[f][:, :],
                out_offset=bass.IndirectOffsetOnAxis(
                    ap=slotL_i[:, f:f + 1], axis=0),
                in_=tid_sb[:, f:f + 1], in_offset=None)
        # merge into idxAll rows 160f+[0:128); sentinel-fill the tail
        sent = p4.tile([128, 9], I32, tag="sent")
        nc.vector.memset(sent[:], N)
        nc.sync.dma_start(
            idxAll[5120:6272, :].rearrange("(f p) c -> p (f c)", p=128),
            sent[:])
        idxL_sb = p4.tile([128, 32], I32, tag="idxL_sb")
        for f in range(32):
            nc.sync.dma_start(idxL_sb[:, f:f + 1], idxL[f][0:128, :])
        nc.sync.dma_start(
            idxAll[0:5120, :].rearrange("(f p) c -> p f c", p=160)[0:128],
            idxL_sb[:].rearrange("p (f c) -> p f c", c=1))
        # level-2: src row of global slot s = 160*f(s) + s - excl[f(s)]
        sg_sb = p4.tile([128, 8], F32, tag="sg_sb")
        nc.sync.dma_start(sg_sb[:], sgrid[:, :])
        io_sb = p4.tile([128, 32], F32, tag="io_sb")
        nc.sync.dma_start(io_sb[:], iota32[:, :])
        ge3 = p4.tile([128, 8 * 32], F32, tag="ge3")
        nc.vector.tensor_tensor(
            out=ge3[:].rearrange("p (j f) -> p j f", f=32),
            in0=sg_sb[:].rearrange("p (j o) -> p j o", o=1)
            .to_broadcast([128, 8, 32]),
            in1=incb[:].rearrange("p (o f) -> p o f", o=1)
            .to_broadcast([128, 8, 32]),
            op=ALU.is_ge)
        fofs = p4.tile([128, 8], F32, tag="fofs")
        nc.vector.reduce_sum(fofs[:],
                             ge3[:].rearrange("p (j f) -> p j f", f=32),
                             axis=mybir.AxisListType.X)
        oh3 = p4.tile([128, 8 * 32], F32, tag="oh3")
        nc.vector.tensor_tensor(
            out=oh3[:].rearrange("p (j f) -> p j f", f=32),
            in0=fofs[:].rearrange("p (j o) -> p j o", o=1)
            .to_broadcast([128, 8, 32]),
            in1=io_sb[:].rearrange("p (o f) -> p o f", o=1)
            .to_broadcast([128, 8, 32]),
            op=ALU.is_equal)
        nc.vector.tensor_tensor(
            out=oh3[:].rearrange("p (j f) -> p j f", f=32),
            in0=oh3[:].rearrange("p (j f) -> p j f", f=32),
            in1=exclb[:].rearrange("p (o f) -> p o f", o=1)
            .to_broadcast([128, 8, 32]),
            op=ALU.mult)
        exclsel = p4.tile([128, 8], F32, tag="exclsel")
        nc.vector.reduce_sum(exclsel[:],
                             oh3[:].rearrange("p (j f) -> p j f", f=32),
                             axis=mybir.AxisListType.X)
        srcf = p4.tile([128, 8], F32, tag="srcf")
        nc.vector.tensor_scalar_mul(srcf[:], fofs[:], 160.0)
        nc.vector.tensor_add(srcf[:], srcf[:], sg_sb[:])
        nc.vector.tensor_tensor(out=srcf[:], in0=srcf[:], in1=exclsel[:],
                                op=ALU.subtract)
        src_i = p4.tile([128, 8], I32, tag="src_i")
        nc.vector.tensor_copy(src_i[:], srcf[:])
        for j in range(8):
            nc.gpsimd.indirect_dma_start(
                out=idx_sb[:, j:j + 1], out_offset=None, in_=idxAll[:, :],
                in_offset=bass.IndirectOffsetOnAxis(
                    ap=src_i[:, j:j + 1], axis=0))
        for j in range(8):
            wrow = p4.tile([128, 16], BF16, tag="wrow")
            nc.gpsimd.indirect_dma_start(
                out=wrow[:], out_offset=None, in_=agout_w[:, :],
                in_offset=bass.IndirectOffsetOnAxis(
                    ap=idx_sb[:, j:j + 1], axis=0))
            wrs = p4.tile([128, 16], F32, tag="wrs")
            nc.vector.tensor_mul(wrs[:], wrow[:], emask_sb[:])
            nc.vector.reduce_sum(wslot[:, j:j + 1], wrs[:],
                                 axis=mybir.AxisListType.X)
        nc.vector.tensor_scalar_mul(wslot[:], wslot[:], 1.0 / WS)

    nc.gpsimd.collective_compute(
        "AllGather", ALU.bypass, replica_groups=RG,
        ins=[agin_h2[:, :].opt()], outs=[agout_h2[0:N, :].opt()])

    # ---------------- Phase 5: expert FFN (fp8 DoubleRow) ----------------
    with tc.tile_pool(name="p5g", bufs=2) as p5g, \
         tc.tile_pool(name="p5", bufs=1) as p5, \
         tc.tile_pool(name="p5at", bufs=5) as p5at, \
         tc.tile_pool(name="p5w", bufs=3) as p5w:
        p5ps_cm = tc.tile_pool(name="p5psA", bufs=2, space="PSUM")
        p5ps = p5ps_cm.__enter__()
        h2cT = p5.tile([128, 8 * CAP], FP8, tag="h2cT")
        for j in range(8):
            hc = p5g.tile([128, D], BF16, tag="hc")
            nc.gpsimd.indirect_dma_start(
                out=hc[:], out_offset=None, in_=agout_h2[:, :],
                in_offset=bass.IndirectOffsetOnAxis(
                    ap=idx_sb[:, j:j + 1], axis=0))
            for dt in range(8):
                pst = p5ps.tile([128, 128], BF16, tag="tp5")
                nc.tensor.transpose(pst[:], hc[:, dt * 128:(dt + 1) * 128],
                                    identb[:])
                nc.vector.tensor_copy(
                    h2cT[:, dt * CAP + j * 128:dt * CAP + (j + 1) * 128],
                    pst[:])
        p5ps_cm.__exit__(None, None, None)
        p5ps_cm2 = tc.tile_pool(name="p5psB", bufs=2, space="PSUM")
        p5ps = p5ps_cm2.__enter__()
        outsb = p5.tile([128, 8 * D], F32, tag="outsb")
        for g in range(4):
            at2s, wp2s = [], []
            for p8 in range(4):
                at2 = p5at.tile([128, 2 * CAP], FP8, tag="at2")
                at2s.append(at2)
                for c2 in range(2):
                    gfc = g * 8 + p8 * 2 + c2
                    wfp = p5w.tile([128, 1024], FP8, tag="wfp")
                    nc.sync.dma_start(
                        wfp[:].rearrange("p (dt m) -> p dt m", m=128),
                        wfc8[:, gfc * 128:(gfc + 1) * 128].rearrange(
                            "(dt p) m -> p dt m", p=128))
                    ps1 = p5ps.tile([128, CAP], F32, tag="ps1")
                    wfp3 = wfp[:].rearrange("p (dt m) -> p dt m", m=128)
                    h2c3 = h2cT[:].rearrange("p (dt m) -> p dt m", m=CAP)
                    for dtp in range(4):
                        for half in range(2):
                            nc.tensor.matmul(
                                ps1[:, half * 512:(half + 1) * 512],
                                wfp3[:, 2 * dtp:2 * dtp + 2, :],
                                h2c3[:, 2 * dtp:2 * dtp + 2,
                                     half * 512:(half + 1) * 512],
                                start=(dtp == 0), stop=(dtp == 3),
                                perf_mode=DR)
                    nc.scalar.activation(at2[:, c2 * CAP:(c2 + 1) * CAP],
                                         ps1[:], AF.Gelu, scale=1.0 / WS)
                wp2 = p5at.tile([128, 2 * D], FP8, tag="wp2")
                gp = g * 4 + p8
                nc.sync.dma_start(
                    wp2[:].rearrange("p (c2 m) -> p c2 m", c2=2),
                    wpj8[gp * 256:(gp + 1) * 256, :].rearrange(
                        "(c2 p) m -> p c2 m", p=128))
                wp2s.append(wp2)
            for tt in range(8):
                ps2 = p5ps.tile([128, D], F32, tag="ps2")
                for p8 in range(4):
                    at3 = at2s[p8][:].rearrange("p (c2 m) -> p c2 m", c2=2)
                    wp3 = wp2s[p8][:].rearrange("p (c2 m) -> p c2 m", c2=2)
                    for half in range(2):
                        nc.tensor.matmul(
                            ps2[:, half * 512:(half + 1) * 512],
                            at3[:, :, tt * 128:(tt + 1) * 128],
                            wp3[:, :, half * 512:(half + 1) * 512],
                            start=(p8 == 0), stop=(p8 == 3),
                            perf_mode=DR)
                od = outsb[:, tt * D:(tt + 1) * D]
                if g == 0:
                    nc.vector.tensor_copy(od, ps2[:])
                else:
                    nc.vector.tensor_add(od, od, ps2[:])
        for tt in range(8):
            sc = p5g.tile([128, D], BF16, tag="sc")
            nc.scalar.activation(sc[:], outsb[:, tt * D:(tt + 1) * D],
                                 AF.Copy, scale=wslot[:, tt:tt + 1])
            nc.sync.dma_start(ffnout[tt * 128:(tt + 1) * 128, :], sc[:])
        p5ps_cm2.__exit__(None, None, None)

    # ---------------- Phase 6: combine + ReduceScatter ----------------
    with tc.tile_pool(name="p6", bufs=3) as p6:
        for f in range(32):
            gb = p6.tile([128, D], BF16, tag="gb")
            nc.gpsimd.indirect_dma_start(
                out=gb[:], out_offset=None, in_=ffnout[:, :],
                in_offset=bass.IndirectOffsetOnAxis(
                    ap=slot_i[:, f:f + 1], axis=0))
            nc.sync.dma_start(rsin[f * 128:(f + 1) * 128, :], gb[:])
        nc.gpsimd.collective_compute(
            "ReduceScatter", ALU.add, replica_groups=RG,
            ins=[rsin[0:N, :].opt()], outs=[rsout[:, :].opt()])

    # ---------------- Phase 7: final assembly ----------------
    with tc.tile_pool(name="p7", bufs=2) as p7:
        lnb = p7.tile([128, D], F32, tag="lnb")
        nc.sync.dma_start(lnb[:], ln2bc[:, :])
        for tt in range(4):
            rs = p7.tile([128, D], BF16, tag="rs")
            nc.sync.dma_start(rs[:], rsout[tt * 128:(tt + 1) * 128, :])
            h2t7 = p7.tile([128, D], BF16, tag="h2t7")
            nc.sync.dma_start(h2t7[:], agin_h2[tt * 128:(tt + 1) * 128, :])
            xm7 = p7.tile([128, D], F32, tag="xm7")
            nc.sync.dma_start(xm7[:], xmid_d[tt * 128:(tt + 1) * 128, :])
            idt = p7.tile([128, D], F32, tag="idt")
            nc.vector.tensor_mul(idt[:], h2t7[:], lnb[:])
            nc.scalar.activation(idt[:], idt[:], AF.Copy,
                                 scale=w8[:, tt:tt + 1])
            nc.vector.tensor_add(idt[:], idt[:], rs[:])
            nc.vector.tensor_add(idt[:], idt[:], xm7[:])
            nc.sync.dma_start(out[tt * 128:(tt + 1) * 128, :], idt[:])
    for pl in (cmp_p, per_p, cst_p, ident_p):
        pl.release()


def _layernorm(nc, pool, xs, d):
    """LN (no weight) on [128, d]; returns (xo, nmean=-mu, rstd)."""
    rsum = pool.tile([128, 1], F32, tag="ln_rsum")
    nc.vector.reduce_sum(rsum[:], xs[:], axis=mybir.AxisListType.X)
    nmean = pool.tile([128, 1], F32, tag="ln_nmean")
    nc.vector.tensor_scalar_mul(nmean[:], rsum[:], -1.0 / d)
    xc = pool.tile([128, d], F32, tag="ln_xc")
    nc.vector.tensor_scalar_add(xc[:], xs[:], nmean[:])
    ssum = pool.tile([128, 1], F32, tag="ln_ssum")
    nc.scalar.activation(xs[:], xc[:], AF.Square, accum_out=ssum[:])
    std = pool.tile([128, 1], F32, tag="ln_std")
    nc.scalar.activation(std[:], ssum[:], AF.Sqrt, bias=nc.eps_sb[:],
                         scale=1.0 / d)
    rstd = pool.tile([128, 1], F32, tag="ln_rstd")
    nc.vector.reciprocal(rstd[:], std[:])
    xo = pool.tile([128, d], F32, tag="ln_xo")
    nc.scalar.activation(xo[:], xc[:], AF.Copy, scale=rstd[:])
    return xo, nmean, rstd


# ---------------------------------------------------------------------------
# host side
# ---------------------------------------------------------------------------

def _host_prep(inputs):
    x = np.asarray(inputs["x"], np.float32).reshape(N, D)
    ln1 = np.asarray(inputs["ln1_w"], np.float64)
    ln2 = np.asarray(inputs["ln2_w"], np.float64)
    wqkv64 = np.asarray(inputs["Wqkv"], np.float64) * ln1[:, None]
    wproj64 = np.asarray(inputs["Wproj"], np.float64)
    wrouter64 = np.asarray(inputs["router_W"], np.float64) * ln2[:, None]
    wqkv = wqkv64.astype(np.float32)
    wproj = wproj64.astype(np.float32)
    wrouter = wrouter64.astype(np.float32)
    srow = wrouter64.sum(0).astype(np.float32).reshape(1, E + 1)
    # Wvp[d, h*9+j] = sum_m Wv[d, 64h+m] * Pr[64h+m, j],  Pr = Wproj@W~r
    Pr = wproj64 @ wrouter64
    wvp = np.zeros((D, H * 9), np.float64)
    for h in range(H):
        wvp[:, h * 9:(h + 1) * 9] = (
            wqkv64[:, 2 * D + h * HD:2 * D + (h + 1) * HD]
            @ Pr[h * HD:(h + 1) * HD, :])
    wvp = wvp.astype(np.float32)
    wfc64 = np.asarray(inputs["W_fc"], np.float64) * ln2[None, :, None]
    wpj64 = np.asarray(inputs["W_pj"], np.float64)
    wfc8 = np.clip(wfc64 * WS, -240, 240).astype(ml_dtypes.float8_e4m3fn)
    wpj8 = np.clip(wpj64 * WS, -240, 240).astype(ml_dtypes.float8_e4m3fn)
    ln2bc = np.broadcast_to(ln2.astype(np.float32), (128, D)).copy()
    tri = (np.arange(128)[:, None] < np.arange(128)[None, :]).astype(
        np.float32)
    tid = (np.arange(128)[:, None] + 128 * np.arange(32)[None, :]).astype(
        np.int32)
    sgrid = (np.arange(128)[:, None] + 128 * np.arange(8)[None, :]).astype(
        np.float32)
    iota32 = np.broadcast_to(
        np.arange(32, dtype=np.float32)[None, :], (128, 32)).copy()
    p = np.arange(128)[:, None]
    q = np.arange(OWN)[None, :]
    dmask = np.concatenate(
        [(q >= 128 * ch + p).astype(np.float32) for ch in range(4)],
        axis=1).astype(ml_dtypes.bfloat16)

    in_maps = []
    for c in range(NC):
        x_c = x[OWN * c:OWN * (c + 1)]
        bi = c % 4
        base = 2048 * (c // 4)
        cols = []
        for ch in range(4):
            cols.append(base + 512 * bi + 128 * ch + np.arange(128))
        for blk in range(bi):
            for ch in range(4):
                cols.append(base + 512 * blk + 128 * ch + np.arange(128))
        while len(cols) < NCH:
            cols.append(np.full(128, N))
        ksel_c = np.stack(cols, axis=1).astype(np.int32)
        em = np.zeros((128, 16), np.float32)
        em[:, c] = 1.0
        in_maps.append({
            "x_own": np.ascontiguousarray(x_c),
            "wqkv": wqkv, "wvp": wvp, "wproj": wproj, "wrouter": wrouter,
            "srow": srow,
            "wfc8": np.ascontiguousarray(wfc8[c]),
            "wpj8": np.ascontiguousarray(wpj8[c]),
            "ln2bc": ln2bc, "tri": tri, "dmask": dmask, "tid": tid,
            "ksel": ksel_c, "emask": em, "sgrid": sgrid, "iota32": iota32,
        })
    return in_maps


def _host_assemble(results):
    return np.concatenate(
        [results[c]["out"] for c in range(NC)], axis=0).reshape(B, T, D)


_NC_CACHE = None


def _get_nc():
    global _NC_CACHE
    if _NC_CACHE is None:
        _NC_CACHE = build_nc()
    return _NC_CACHE


def kernel(**inputs):
    from concourse import bass_utils
    nc = _get_nc()
    in_maps = _host_prep(inputs)
    res = bass_utils.run_bass_kernel_spmd(nc, in_maps,
                                          core_ids=list(range(NC)))
    return _host_assemble(res.results)


if __name__ == "__main__":
    nc = build_nc()
    print("built ok")



# revision 17
# speedup vs baseline: 1.6265x; 1.6265x over previous
"""Trainium2 Bass kernel for nn_Block (attention + soft top-2 MoE), 8-core SPMD.

v3: - Subgroup (per-batch) AllGathers for k/v, issued mid-phase-1; k shipped
      pre-transposed bf16, v bf16; natural chunk order with a full per-core
      causal mask input (no ksel indirection).
    - Matmul-based routing compaction (one-hot x [pid, filled, w] rhs), all
      in SBUF; slot table bounced through one small DRAM tensor.
    - FFN outputs scattered straight into a pre-zeroed ReduceScatter input
      (no ffnout gather bounce).
    - wproj loaded once + prefetched; fp8 FFN weights host-pretiled and
      prefetched; all-at2-resident FFN (no DVE accumulation adds).
"""

import sys

if "/opt/trn_rl_repo" not in sys.path:
    sys.path.insert(0, "/opt/trn_rl_repo")

import numpy as np
import ml_dtypes

import concourse.bass as bass
import concourse.mybir as mybir
import concourse.tile as tile
from concourse import bacc
from concourse.masks import make_identity

F32 = mybir.dt.float32
F32R = mybir.dt.float32r
BF16 = mybir.dt.bfloat16
FP8 = mybir.dt.float8e4
I32 = mybir.dt.int32
AF = mybir.ActivationFunctionType
ALU = mybir.AluOpType
DR = mybir.MatmulPerfMode.DoubleRow

B, T, D = 2, 2048, 1024
H, HD = 16, 64
E = 8
NC = 8
N = B * T
OWN = N // NC              # 512
NCH = 16
DFF = 4 * D
CAP = 1024
VA = 74                    # 64 v + 1 ones(den) + 9 VP
EPS = 1e-5
RG8 = [list(range(NC))]
RG4 = [[0, 1, 2, 3], [4, 5, 6, 7]]
WS = 32.0                  # fp8 weight pre-scale


def r32(x):
    return x.bitcast(F32R)


def f32(x):
    return x.bitcast(F32)


def build_nc():
    nc = bacc.Bacc("TRN2", target_bir_lowering=False, debug=False,
                   num_devices=NC)

    x_own = nc.dram_tensor("x_own", [OWN, D], F32, kind="ExternalInput")
    wqkv = nc.dram_tensor("wqkv", [D, 3 * D], F32R, kind="ExternalInput")
    wvp_t = nc.dram_tensor("wvp_t", [128, 8 * H * 9], F32,
                           kind="ExternalInput")
    wproj = nc.dram_tensor("wproj", [D, D], F32R, kind="ExternalInput")
    wr_t = nc.dram_tensor("wr_t", [128, 8 * (E + 1)], F32,
                          kind="ExternalInput")
    srow = nc.dram_tensor("srow", [1, E + 1], F32, kind="ExternalInput")
    wfc8_t = nc.dram_tensor("wfc8_t", [128, 8 * DFF], FP8,
                            kind="ExternalInput")
    wpj8_t = nc.dram_tensor("wpj8_t", [128, 32 * D], FP8,
                            kind="ExternalInput")
    ln2bc = nc.dram_tensor("ln2bc", [128, D], F32, kind="ExternalInput")
    tri = nc.dram_tensor("tri", [128, 128], F32R, kind="ExternalInput")
    tri32x = nc.dram_tensor("tri32x", [32, 64], F32R, kind="ExternalInput")
    dmask16 = nc.dram_tensor("dmask16", [128, NCH * OWN], BF16,
                             kind="ExternalInput")
    emask = nc.dram_tensor("emask", [128, 16], F32, kind="ExternalInput")
    sgrid = nc.dram_tensor("sgrid", [128, 8], F32, kind="ExternalInput")
    iota32 = nc.dram_tensor("iota32", [128, 32], F32, kind="ExternalInput")
    iotap = nc.dram_tensor("iotap", [128, 1], F32, kind="ExternalInput")
    iota128r = nc.dram_tensor("iota128r", [128, 128], F32,
                              kind="ExternalInput")
    out = nc.dram_tensor("out", [OWN, D], F32, kind="ExternalOutput")

    agin_kT = nc.dram_tensor("agin_kT", [128, 8 * OWN], BF16)
    agout_kT = nc.dram_tensor("agout_kT", [512, 8 * OWN], BF16)
    agin_v = nc.dram_tensor("agin_v", [OWN, H * VA], BF16)
    agout_v = nc.dram_tensor("agout_v", [4 * OWN, H * VA], BF16)
    agin_h2 = nc.dram_tensor("agin_h2", [OWN, D], BF16)
    agout_h2 = nc.dram_tensor("agout_h2", [N + 64, D], BF16,
                              addr_space="Shared")
    agin_w = nc.dram_tensor("agin_w", [OWN, 16], BF16)
    agout_w = nc.dram_tensor("agout_w", [N, 16], BF16, addr_space="Shared")
    idxw_d = nc.dram_tensor("idxw_d", [32 * 128 + 128, 2], I32)
    yt2_d = nc.dram_tensor("yt2_d", [128, 8 * OWN], F32R)
    rsin = nc.dram_tensor("rsin", [N + 64, D], BF16)
    rsout = nc.dram_tensor("rsout", [OWN, D], BF16)

    with tile.TileContext(nc) as tc:
        build_body(nc, tc, locals())
    nc.compile()
    return nc


def build_body(nc, tc, t):
    x_own, wqkv, wvp_t, wproj, wr_t, srow = (
        t["x_own"], t["wqkv"], t["wvp_t"], t["wproj"], t["wr_t"], t["srow"])
    wfc8_t, wpj8_t, ln2bc, tri, tri32x, dmask16, emask = (
        t["wfc8_t"], t["wpj8_t"], t["ln2bc"], t["tri"], t["tri32x"],
        t["dmask16"], t["emask"])
    sgrid, iota32, iotap, iota128r = (
        t["sgrid"], t["iota32"], t["iotap"], t["iota128r"])
    agin_kT, agout_kT, agin_v, agout_v = (
        t["agin_kT"], t["agout_kT"], t["agin_v"], t["agout_v"])
    agin_h2, agout_h2, agin_w, agout_w = (
        t["agin_h2"], t["agout_h2"], t["agin_w"], t["agout_w"])
    idxw_d, yt2_d, rsin, rsout, out = (
        t["idxw_d"], t["yt2_d"], t["rsin"], t["rsout"], t["out"])

    ident_p = tc.alloc_tile_pool(name="ident", bufs=1)
    ident = ident_p.tile([128, 128], F32)
    make_identity(nc, ident[:])
    identb = ident_p.tile([128, 128], BF16)
    nc.vector.tensor_copy(identb[:], ident[:])

    cst_p = tc.alloc_tile_pool(name="cst", bufs=1)
    emask_sb = cst_p.tile([128, 16], F32)
    nc.sync.dma_start(emask_sb[:], emask[:, :])
    ones_cf = cst_p.tile([128, 1], F32)
    nc.vector.memset(ones_cf[:], 1.0)
    ones_c = cst_p.tile([128, 1], F32R)
    nc.vector.tensor_copy(ones_c[:], ones_cf[:])
    ones_rf = cst_p.tile([1, 128], F32)
    nc.vector.memset(ones_rf[:], 1.0)
    ones_r = cst_p.tile([1, 128], F32R)
    nc.vector.tensor_copy(ones_r[:], ones_rf[:])
    eps_sb = cst_p.tile([128, 1], F32)
    nc.vector.memset(eps_sb[:], EPS)
    nc.eps_sb = eps_sb
    w8 = cst_p.tile([128, 4], F32)              # identity-expert weight
    srow_sb = cst_p.tile([1, E + 1], F32)
    nc.sync.dma_start(srow_sb[:], srow[:, :])

    # residuals kept on-chip phase3 -> phase7
    res_p = tc.alloc_tile_pool(name="res", bufs=1)
    xmsb = res_p.tile([128, 4 * D], F32)
    h2sb = res_p.tile([128, 4 * D], BF16)

    # persistent SBUF across phases 1-3 (released before FFN)
    per_p = tc.alloc_tile_pool(name="per", bufs=1)
    qT = per_p.tile([128, 8 * OWN], BF16)        # [d, q] scaled 1/8 (host)
    ndg = [per_p.tile([128, OWN], F32, name=f"ndg{i}") for i in range(4)]
    ndT = per_p.tile([128, 4 * OWN], F32)        # transposed nd per tt
    xr0 = per_p.tile([E + 1, OWN], F32)          # wr^T @ x^T, [9, tok]
    xrT = per_p.tile([E + 1, OWN], F32)          # + s (x) (-mu2), [9, tok]
    nmean4 = per_p.tile([128, 4], F32)           # -mu2 per tt (phase 3)
    rstd4 = per_p.tile([128, 4], F32)
    muT = per_p.tile([1, OWN], F32)              # mu2 row (phase 3)


    # attention working set: allocated for phases 1-3, loads overlap phase 1
    p2big_cm = tc.tile_pool(name="p2big", bufs=1)
    p2 = p2big_cm.__enter__()
    kT = p2.tile([128, 8 * 2048], BF16, name="kT")
    vall = p2.tile([128, NCH * H * VA], BF16, name="vall")

    # ---------------- Phase 1: LN1 + QKV + VP + AllGather(kT, v) ----------
    with tc.tile_pool(name="p1a", bufs=1) as p1a, \
         tc.tile_pool(name="p1n", bufs=1) as p1n, \
         tc.tile_pool(name="p1", bufs=1) as p1, \
         tc.tile_pool(name="p1w", bufs=2) as p1w:
        xlnT = p1a.tile([128, 8 * OWN], F32R, tag="xlnT")
        nm1 = p1a.tile([128, 4], F32, tag="nm1")
        rs1 = p1a.tile([128, 4], F32, tag="rs1")
        p1ps_cm = tc.tile_pool(name="p1psT", bufs=2, space="PSUM")
        p1ps = p1ps_cm.__enter__()
        for tt in range(4):
            xs = p1.tile([128, D], F32, tag="xs")
            nc.sync.dma_start(xs[:], x_own[tt * 128:(tt + 1) * 128, :])
            xln, nmean, rstd = _layernorm(nc, p1n, xs[:], D)
            nc.vector.tensor_copy(nm1[:, tt:tt + 1], nmean[:])
            nc.vector.tensor_copy(rs1[:, tt:tt + 1], rstd[:])
            for dt in range(8):
                pst = p1ps.tile([128, 128], F32, tag="tp")
                nc.tensor.transpose(pst[:], xln[:, dt * 128:(dt + 1) * 128],
                                    ident[:])
                nc.vector.tensor_copy(
                    xlnT[:, dt * OWN + tt * 128: dt * OWN + (tt + 1) * 128],
                    pst[:])
        p1ps_cm.__exit__(None, None, None)
        # ---- k first (feeds AG(kT) early) ----
        p1psK_cm = tc.tile_pool(name="p1psK", bufs=1, space="PSUM")
        p1psK = p1psK_cm.__enter__()
        pk0 = p1psK.tile([128, D], F32, tag="pk0")
        pk1 = p1psK.tile([128, D], F32, tag="pk1")
        pk2 = p1psK.tile([128, D], F32, tag="pk2")
        pk3 = p1psK.tile([128, D], F32, tag="pk3")
        pks = [pk0, pk1, pk2, pk3]
        for dt in range(8):
            wp = p1w.tile([128, D], F32R, tag="w")
            nc.sync.dma_start(wp[:], wqkv[dt * 128:(dt + 1) * 128, D:2 * D])
            for tt in range(4):
                for half in range(2):
                    nc.tensor.matmul(
                        pks[tt][:, half * 512:(half + 1) * 512],
                        r32(xlnT[:, dt * OWN + tt * 128:
                                 dt * OWN + (tt + 1) * 128]),
                        r32(wp[:, half * 512:(half + 1) * 512]),
                        start=(dt == 0), stop=(dt == 7))
        ksb = p1a.tile([128, 4 * D], BF16, tag="ksb")
        for tt in range(4):
            nc.vector.tensor_copy(ksb[:, tt * D:(tt + 1) * D], pks[tt][:])
        p1psK_cm.__exit__(None, None, None)
        p1psT2_cm = tc.tile_pool(name="p1psT2", bufs=2, space="PSUM")
        p1psT2 = p1psT2_cm.__enter__()
        kTo = p1a.tile([128, 8 * OWN], BF16, tag="kTo")
        for tt in range(4):
            for dt in range(8):
                pst = p1psT2.tile([128, 128], BF16, tag="tpk")
                nc.tensor.transpose(
                    pst[:], ksb[:, tt * D + dt * 128:tt * D + (dt + 1) * 128],
                    identb[:])
                nc.vector.tensor_copy(
                    kTo[:, dt * OWN + tt * 128:dt * OWN + (tt + 1) * 128],
                    pst[:])
        nc.sync.dma_start(agin_kT[:, :], kTo[:])
        p1psT2_cm.__exit__(None, None, None)
        nc.gpsimd.collective_compute(
            "AllGather", ALU.bypass, replica_groups=RG4,
            ins=[agin_kT[:, :].opt()], outs=[agout_kT[:, :].opt()])
        # ---- q (dt-outer, 8 psum accumulators) ----
        p1psQ_cm = tc.tile_pool(name="p1psQ", bufs=1, space="PSUM")
        p1psQ = p1psQ_cm.__enter__()
        psqs = []
        for m8 in range(8):
            psq = p1psQ.tile([128, OWN], F32, tag=f"psq{m8}", name=f"psq{m8}")
            psqs.append(psq)
        for dt in range(8):
            wq = p1w.tile([128, D], F32R, tag="w")
            nc.sync.dma_start(wq[:], wqkv[dt * 128:(dt + 1) * 128, 0:D])
            for mc in range(8):
                nc.tensor.matmul(
                    psqs[mc][:], r32(wq[:, mc * 128:(mc + 1) * 128]),
                    r32(xlnT[:, dt * OWN:(dt + 1) * OWN]),
                    start=(dt == 0), stop=(dt == 7))
        for mc in range(8):
            nc.vector.tensor_copy(qT[:, mc * OWN:(mc + 1) * OWN],
                                  psqs[mc][:])
        p1psQ_cm.__exit__(None, None, None)
        # ---- VP staging ----
        p1psP_cm = tc.tile_pool(name="p1psP", bufs=2, space="PSUM")
        p1psP = p1psP_cm.__enter__()
        wvp_sb = p1a.tile([128, 8 * H * 9], F32, tag="wvp")
        nc.sync.dma_start(wvp_sb[:], wvp_t[:, :])
        vpsb = p1a.tile([128, 4 * H * 9], F32, tag="vpsb")
        for tt in range(4):
            pvp = p1psP.tile([128, H * 9], F32, tag="pvp")
            for dt in range(8):
                nc.tensor.matmul(
                    pvp[:],
                    f32(xlnT[:, dt * OWN + tt * 128:
                             dt * OWN + (tt + 1) * 128]),
                    wvp_sb[:, dt * H * 9:(dt + 1) * H * 9],
                    start=(dt == 0), stop=(dt == 7))
            nc.vector.tensor_copy(vpsb[:, tt * H * 9:(tt + 1) * H * 9],
                                  pvp[:])
        p1psP_cm.__exit__(None, None, None)
        # ---- v ----
        p1psV_cm = tc.tile_pool(name="p1psV", bufs=1, space="PSUM")
        p1psV = p1psV_cm.__enter__()
        pv0 = p1psV.tile([128, D], F32, tag="pv0")
        pv1 = p1psV.tile([128, D], F32, tag="pv1")
        pv2 = p1psV.tile([128, D], F32, tag="pv2")
        pv3 = p1psV.tile([128, D], F32, tag="pv3")
        pvs = [pv0, pv1, pv2, pv3]
        for dt in range(8):
            wp = p1w.tile([128, D], F32R, tag="w")
            nc.sync.dma_start(wp[:],
                              wqkv[dt * 128:(dt + 1) * 128, 2 * D:3 * D])
            for tt in range(4):
                for half in range(2):
                    nc.tensor.matmul(
                        pvs[tt][:, half * 512:(half + 1) * 512],
                        r32(xlnT[:, dt * OWN + tt * 128:
                                 dt * OWN + (tt + 1) * 128]),
                        r32(wp[:, half * 512:(half + 1) * 512]),
                        start=(dt == 0), stop=(dt == 7))
        for tt in range(4):
            vs = p1.tile([128, H * VA], BF16, tag="vo")
            vv = vs[:].rearrange("p (h c) -> p h c", c=VA)
            nc.vector.memset(vv[:, :, 64:65], 1.0)
            nc.vector.tensor_copy(
                vv[:, :, 0:64],
                pvs[tt][:].rearrange("p (h c) -> p h c", c=64))
            nc.vector.tensor_copy(
                vv[:, :, 65:74],
                vpsb[:, tt * H * 9:(tt + 1) * H * 9].rearrange(
                    "p (h c) -> p h c", c=9))
            nc.sync.dma_start(agin_v[tt * 128:(tt + 1) * 128, :], vs[:])
        p1psV_cm.__exit__(None, None, None)
        nc.gpsimd.collective_compute(
            "AllGather", ALU.bypass, replica_groups=RG4,
            ins=[agin_v[:, :].opt()], outs=[agout_v[:, :].opt()])
        # pre-zero the ReduceScatter input + the h2 sentinel pad (off-path)
        zb4 = p1a.tile([128, 2 * D], BF16, tag="zb4")
        nc.vector.memset(zb4[:], 0.0)
        for f8 in range(16):
            nc.sync.dma_start(
                rsin[f8 * 256:(f8 + 1) * 256, :].rearrange(
                    "(g p) c -> p g c", p=128),
                zb4[:].rearrange("p (g c) -> p g c", c=D))
        nc.sync.dma_start(rsin[N:N + 64, :], zb4[0:64, 0:D])
        nc.sync.dma_start(agout_h2[N:N + 64, :], zb4[0:64, 0:D])
        # ---- xr0 = (wr^T @ xln^T) * sig1 + srow^T (x) mu1 ----
        p1psR_cm = tc.tile_pool(name="p1psR", bufs=1, space="PSUM")
        p1psR = p1psR_cm.__enter__()
        sigrow = p1a.tile([1, OWN], F32, tag="sigrow")
        murow = p1a.tile([1, OWN], F32, tag="murow")
        sig4 = p1a.tile([128, 4], F32, tag="sig4")
        nc.vector.reciprocal(sig4[:], rs1[:])
        mu4 = p1a.tile([128, 4], F32, tag="mu4")
        nc.vector.tensor_scalar_mul(mu4[:], nm1[:], -1.0)
        for tt in range(4):
            pss = p1psR.tile([1, 128], F32, tag="pss")
            nc.tensor.transpose(pss[:], sig4[:, tt:tt + 1], ident[:])
            nc.vector.tensor_copy(sigrow[:, tt * 128:(tt + 1) * 128], pss[:])
            psm = p1psR.tile([1, 128], F32, tag="psm")
            nc.tensor.transpose(psm[:], mu4[:, tt:tt + 1], ident[:])
            nc.vector.tensor_copy(murow[:, tt * 128:(tt + 1) * 128], psm[:])
        wr_sb = p1a.tile([128, 8 * (E + 1)], F32, tag="wr")
        nc.sync.dma_start(wr_sb[:], wr_t[:, :])
        pxr = p1psR.tile([E + 1, OWN], F32, tag="pxr")
        for dt in range(8):
            nc.tensor.matmul(
                pxr[:], wr_sb[:, dt * (E + 1):(dt + 1) * (E + 1)],
                f32(xlnT[:, dt * OWN:(dt + 1) * OWN]),
                start=(dt == 0), stop=(dt == 7))
        psig = p1psR.tile([E + 1, OWN], F32, tag="psig")
        nc.tensor.matmul(psig[:], ones_rf[:, 0:E + 1], sigrow[:],
                         start=True, stop=True)
        pmu = p1psR.tile([E + 1, OWN], F32, tag="pmu")
        nc.tensor.matmul(pmu[:], srow_sb[:], murow[:], start=True, stop=True)
        nc.vector.tensor_copy(xr0[:], pxr[:])
        nc.vector.tensor_mul(xr0[:], xr0[:], psig[:])
        nc.vector.tensor_add(xr0[:], xr0[:], pmu[:])
        p1psR_cm.__exit__(None, None, None)

    # wproj prefetch (used in phase 3) + yT2 staging
    mid_cm = tc.tile_pool(name="mid", bufs=1)
    mid_p = mid_cm.__enter__()
    wproj_sb = mid_p.tile([128, 8 * D], F32R, name="wproj_sb")
    for dt in range(8):
        nc.sync.dma_start(wproj_sb[:, dt * D:(dt + 1) * D],
                          wproj[dt * 128:(dt + 1) * 128, :])

    # ---------------- Phase 2: attention ----------------
    with tc.tile_pool(name="p2c", bufs=1) as p2c, \
         tc.tile_pool(name="p2s", bufs=3) as p2s:
        dm16 = p2c.tile([128, NCH * OWN], BF16, tag="dm16")
        nc.sync.dma_start(dm16[:], dmask16[:, :])
        for dt in range(8):
            nc.sync.dma_start(
                kT[:, dt * 2048:(dt + 1) * 2048].rearrange(
                    "p (c t) -> p c t", t=OWN),
                agout_kT[:, dt * OWN:(dt + 1) * OWN].rearrange(
                    "(c p) t -> p c t", p=128))
        for q4 in range(4):
            nc.sync.dma_start(
                vall[:, q4 * 4 * H * VA:(q4 + 1) * 4 * H * VA].rearrange(
                    "p (ch m) -> p ch m", m=H * VA),
                agout_v[q4 * 512:(q4 + 1) * 512, :].rearrange(
                    "(ch p) m -> p ch m", p=128))
        p2ps_cm = tc.tile_pool(name="p2psB", bufs=2, space="PSUM")
        p2ps = p2ps_cm.__enter__()
        p2psc_cm = tc.tile_pool(name="p2psC", bufs=1, space="PSUM")
        p2psc = p2psc_cm.__enter__()
        p2psy_cm = tc.tile_pool(name="p2psY", bufs=1, space="PSUM")
        p2psy = p2psy_cm.__enter__()
        for hp in range(8):                       # head pairs (dt = hp)
            ypss = []
            for hh in range(2):
                yps = p2psy.tile([VA, OWN], F32, tag=f"yps{hh}")
                ypss.append(yps)
            for ch in range(NCH):
                psc = p2ps.tile([128, 2 * OWN], F32, tag="psc")
                for hh in range(2):
                    ph = hh * 64
                    nc.tensor.matmul(
                        psc[:, hh * OWN:(hh + 1) * OWN],
                        kT[ph:ph + 64, hp * 2048 + ch * 128:
                           hp * 2048 + (ch + 1) * 128],
                        qT[ph:ph + 64, hp * OWN:(hp + 1) * OWN],
                        start=True, stop=True)
                ex = p2s.tile([128, 2 * OWN], BF16, tag="ex")
                nc.scalar.activation(ex[:], psc[:], AF.Exp)
                nc.vector.tensor_tensor(
                    out=ex[:].rearrange("p (g m) -> p g m", g=2),
                    in0=ex[:].rearrange("p (g m) -> p g m", g=2),
                    in1=dm16[:, ch * OWN:(ch + 1) * OWN].rearrange(
                        "p (g m) -> p g m", g=1).to_broadcast(
                        [128, 2, OWN]),
                    op=ALU.mult)
                for hh in range(2):
                    h = hp * 2 + hh
                    nc.tensor.matmul(
                        ypss[hh][:],
                        vall[:, ch * H * VA + h * VA:
                             ch * H * VA + (h + 1) * VA],
                        ex[:, hh * OWN:(hh + 1) * OWN],
                        start=(ch == 0), stop=(ch == NCH - 1))
            for hh in range(2):
                h = hp * 2 + hh
                ph = hh * 64
                yps = ypss[hh]
                rin = p2s.tile([1, OWN], F32R, tag="rin")
                with nc.allow_low_precision(reason="f32r rhs for bcast mm"):
                    nc.vector.reciprocal(rin[:], yps[64:65, :])
                pbc = p2psc.tile([64, OWN], F32, tag="pbc")
                nc.tensor.matmul(pbc[:], r32(ones_r[:, 0:64]), r32(rin[:]),
                                 start=True, stop=True)
                pbs = p2s.tile([64, OWN], F32, tag="pbs")
                nc.vector.tensor_copy(pbs[:], pbc[:])
                nc.vector.tensor_tensor(
                    out=pbs[:], in0=yps[0:64, :], in1=pbs[:], op=ALU.mult)
                nc.sync.dma_start(
                    yt2_d[ph:ph + 64, hp * OWN:(hp + 1) * OWN],
                    r32(pbs[:]))
                # stack den+num rows for the router path (32-part aligned)
                grp, slot = h // 4, h % 4
                nc.vector.tensor_copy(
                    ndg[grp][32 * slot:32 * slot + 10, :], yps[64:74, :])
        p2psy_cm.__exit__(None, None, None)
        p2psc_cm.__exit__(None, None, None)
        p2ps_cm.__exit__(None, None, None)
        # transpose nd stacks to token-major [128, 4*128] per tt
        p2psd_cm = tc.tile_pool(name="p2psD", bufs=2, space="PSUM")
        p2psd = p2psd_cm.__enter__()
        for grp in range(4):
            for tt in range(4):
                pst = p2psd.tile([128, 128], F32, tag="tpn")
                nc.tensor.transpose(
                    pst[:], ndg[grp][:, tt * 128:(tt + 1) * 128], ident[:])
                nc.vector.tensor_copy(
                    ndT[:, tt * 512 + grp * 128:tt * 512 + (grp + 1) * 128],
                    pst[:])
        p2psd_cm.__exit__(None, None, None)

    # ------------- Phase 3: proj + residual + LN2 + logits + router ------
    with tc.tile_pool(name="p3", bufs=2) as p3, \
         tc.tile_pool(name="p3n", bufs=1) as p3n:
        p3ps_cm = tc.tile_pool(name="p3psA", bufs=2, space="PSUM")
        p3ps = p3ps_cm.__enter__()
        yT2 = mid_p.tile([128, 8 * OWN], F32R, tag="yT2")
        nc.sync.dma_start(yT2[:], yt2_d[:, :])
        for tt in range(4):
            pp = p3ps.tile([128, D], F32, tag="pp")
            for dt in range(8):
                for half in range(2):
                    nc.tensor.matmul(
                        pp[:, half * 512:(half + 1) * 512],
                        r32(yT2[:, dt * OWN + tt * 128:
                                dt * OWN + (tt + 1) * 128]),
                        r32(wproj_sb[:, dt * D + half * 512:
                                     dt * D + (half + 1) * 512]),
                        start=(dt == 0), stop=(dt == 7))
            xot = p3.tile([128, D], F32, tag="xot")
            nc.sync.dma_start(xot[:], x_own[tt * 128:(tt + 1) * 128, :])
            xmt = xmsb[:, tt * D:(tt + 1) * D]
            nc.vector.tensor_add(xmt, xot[:], pp[:])
            h2t, nmean, rstd = _layernorm(nc, p3n, xmt, D)
            nc.vector.tensor_copy(nmean4[:, tt:tt + 1], nmean[:])
            nc.vector.tensor_copy(rstd4[:, tt:tt + 1], rstd[:])
            h2b = h2sb[:, tt * D:(tt + 1) * D]
            nc.vector.tensor_copy(h2b, h2t[:])
            nc.sync.dma_start(agin_h2[tt * 128:(tt + 1) * 128, :], h2b)
            # -mu2^T strip via transpose
            psm = p3ps.tile([1, 128], F32, tag="psm2")
            nc.tensor.transpose(psm[:], nmean4[:, tt:tt + 1], ident[:])
            nc.vector.tensor_copy(muT[:, tt * 128:(tt + 1) * 128], psm[:])
        # xrT = xr0 + s^T (x) (-mu2^T)
        pxr2 = p3ps.tile([E + 1, OWN], F32, tag="pxr2")
        nc.tensor.matmul(pxr2[:], srow_sb[:], muT[:], start=True, stop=True)
        nc.vector.tensor_add(xrT[:], xr0[:], pxr2[:])
        p3ps_cm.__exit__(None, None, None)
        p3ps_cm2 = tc.tile_pool(name="p3psB", bufs=2, space="PSUM")
        p3ps = p3ps_cm2.__enter__()
        for tt in range(4):
            pxt = p3ps.tile([128, E + 1], F32, tag="pxt")
            nc.tensor.transpose(pxt[:], xrT[:, tt * 128:(tt + 1) * 128],
                                ident[0:E + 1, 0:E + 1])
            # per-head num*recip(den), summed over heads
            lt = p3.tile([128, E + 1], F32, tag="lt")
            nc.vector.tensor_copy(lt[:], pxt[:])
            for grp in range(4):
                ndt = ndT[:, tt * 512 + grp * 128:tt * 512 + (grp + 1) * 128]
                nd3 = ndt.rearrange("p (h c) -> p h c", c=32)
                rec = p3.tile([128, 4], F32, tag="rec")
                nc.vector.reciprocal(
                    rec[:], nd3[:, :, 0:1].rearrange("p h c -> p (h c)"))
                sc8 = p3.tile([128, 36], F32, tag="sc8")
                nc.vector.tensor_tensor(
                    out=sc8[:].rearrange("p (j h) -> p h j", h=4),
                    in0=nd3[:, :, 1:10],
                    in1=rec[:].rearrange("p (h c) -> p h c", c=1)
                    .to_broadcast([128, 4, 9]),
                    op=ALU.mult)
                ssum = p3.tile([128, E + 1], F32, tag="ssum")
                nc.vector.reduce_sum(
                    ssum[:], sc8[:].rearrange("p (j h) -> p j h", h=4),
                    axis=mybir.AxisListType.X)
                nc.vector.tensor_add(lt[:], lt[:], ssum[:])
            nc.vector.tensor_scalar_mul(lt[:], lt[:], rstd4[:, tt:tt + 1])
            # softmax + top-2 weights on [128, 9]
            rmax = p3.tile([128, 1], F32, tag="rmax")
            nc.vector.reduce_max(rmax[:], lt[:], axis=mybir.AxisListType.X)
            nrm = p3.tile([128, 1], F32, tag="nrm")
            nc.vector.tensor_scalar_mul(nrm[:], rmax[:], -1.0)
            prob = p3.tile([128, E + 1], F32, tag="prob")
            sume = p3.tile([128, 1], F32, tag="sume")
            nc.scalar.activation(prob[:], lt[:], AF.Exp, bias=nrm[:],
                                 accum_out=sume[:])
            rinv = p3.tile([128, 1], F32, tag="rinv")
            nc.vector.reciprocal(rinv[:], sume[:])
            nc.scalar.activation(prob[:], prob[:], AF.Copy, scale=rinv[:])
            m1 = p3.tile([128, 1], F32, tag="m1")
            nc.vector.reduce_max(m1[:], prob[:], axis=mybir.AxisListType.X)
            eq = p3.tile([128, E + 1], F32, tag="eq")
            nc.vector.tensor_tensor(
                out=eq[:], in0=prob[:], in1=m1[:].to_broadcast([128, E + 1]),
                op=ALU.is_equal)
            pm = p3.tile([128, E + 1], F32, tag="pm")
            nc.vector.tensor_scalar_mul(pm[:], eq[:], -2.0)
            nc.vector.tensor_add(pm[:], pm[:], prob[:])
            m2 = p3.tile([128, 1], F32, tag="m2")
            nc.vector.reduce_max(m2[:], pm[:], axis=mybir.AxisListType.X)
            ge = p3.tile([128, E + 1], F32, tag="ge")
            nc.vector.tensor_tensor(
                out=ge[:], in0=prob[:], in1=m2[:].to_broadcast([128, E + 1]),
                op=ALU.is_ge)
            w16 = p3.tile([128, 16], F32, tag="w16")
            nc.vector.memset(w16[:], 0.0)
            nc.vector.tensor_mul(w16[:, 0:E + 1], prob[:], ge[:])
            nc.vector.tensor_copy(w8[:, tt:tt + 1], w16[:, E:E + 1])
            w16b = p3.tile([128, 16], BF16, tag="w16b")
            nc.vector.tensor_copy(w16b[:], w16[:])
            nc.sync.dma_start(agin_w[tt * 128:(tt + 1) * 128, :], w16b[:])
        p3ps_cm2.__exit__(None, None, None)
    mid_cm.__exit__(None, None, None)
    p2big_cm.__exit__(None, None, None)
    per_p.release()
    nc.gpsimd.collective_compute(
        "AllGather", ALU.bypass, replica_groups=RG8,
        ins=[agin_w[:, :].opt()], outs=[agout_w[:, :].opt()])
    nc.gpsimd.collective_compute(
        "AllGather", ALU.bypass, replica_groups=RG8,
        ins=[agin_h2[:, :].opt()], outs=[agout_h2[0:N, :].opt()])

    # ---------------- Phase 4: routing compaction (matmul-based) ---------
    cmp_p = tc.alloc_tile_pool(name="cmp", bufs=1)
    idx8 = cmp_p.tile([128, 8], I32)
    wslot = cmp_p.tile([128, 8], F32)
    # FFN weight prefetch (phase 5) — issue before P4 math so DMA overlaps
    p5w_p = tc.alloc_tile_pool(name="p5w", bufs=1)
    wfc_sb = p5w_p.tile([128, 8 * DFF], FP8)
    nc.sync.dma_start(wfc_sb[:], wfc8_t[:, :])
    wpj_sb = p5w_p.tile([128, 32 * D], FP8)
    nc.sync.dma_start(wpj_sb[:], wpj8_t[:, :])
    with tc.tile_pool(name="p4", bufs=1) as p4, \
         tc.tile_pool(name="p4ps", bufs=1, space="PSUM") as p4ps:
        io32 = p4.tile([128, 32], F32, tag="io32")
        nc.sync.dma_start(io32[:], iota32[:, :])
        iop = p4.tile([128, 1], F32, tag="iop")
        nc.sync.dma_start(iop[:], iotap[:, :])
        io128 = p4.tile([128, 128], F32, tag="io128")
        nc.sync.dma_start(io128[:], iota128r[:, :])
        tri_sb = p4.tile([128, 128], F32R, tag="tri")
        nc.sync.dma_start(tri_sb[:], tri[:, :])
        sg_sb = p4.tile([128, 8], F32, tag="sg")
        nc.sync.dma_start(sg_sb[:], sgrid[:, :])
        wfull = p4.tile([128, 32, 16], BF16, tag="wfull")
        nc.sync.dma_start(
            wfull[:], agout_w[:, :].rearrange("(f p) c -> p f c", p=128))
        wsel = p4.tile([128, 32, 16], F32, tag="wsel")
        nc.vector.tensor_tensor(
            out=wsel[:], in0=wfull[:],
            in1=emask_sb[:].rearrange("p (o c) -> p o c", o=1).to_broadcast(
                [128, 32, 16]),
            op=ALU.mult)
        wcol = p4.tile([128, 32], F32, tag="wcol")
        nc.vector.reduce_sum(wcol[:], wsel[:], axis=mybir.AxisListType.X)
        g01 = p4.tile([128, 32], F32R, tag="g01")
        nc.vector.tensor_scalar(out=g01[:], in0=wcol[:], scalar1=0.0,
                                scalar2=None, op0=ALU.is_gt)
        # chunk counts -> excl/incl prefix rows -> broadcast
        pcs = p4ps.tile([1, 32], F32, tag="pcs")
        nc.tensor.matmul(pcs[:], r32(ones_c[:]), r32(g01[:]), start=True,
                         stop=True)
        csum = p4.tile([1, 32], F32, tag="csum")
        nc.vector.tensor_copy(csum[:], pcs[:])
        pfx0 = p4.tile([1, 32], F32, tag="pfx0")
        pfx1 = p4.tile([1, 32], F32, tag="pfx1")
        pfx = [pfx0, pfx1]
        cur = csum
        for i, sh in enumerate([1, 2, 4, 8, 16]):
            nxt = pfx[i % 2]
            nc.vector.tensor_add(nxt[:, sh:32], cur[:, sh:32],
                                 cur[:, 0:32 - sh])
            nc.vector.tensor_copy(nxt[:, 0:sh], cur[:, 0:sh])
            cur = nxt
        # cur = inclusive prefix; build [excl | incl] f32r row
        exi = p4.tile([1, 64], F32, tag="exi")
        nc.vector.memset(exi[:, 0:1], 0.0)
        nc.vector.tensor_copy(exi[:, 1:32], cur[:, 0:31])
        nc.vector.tensor_copy(exi[:, 32:64], cur[:])
        exir = p4.tile([1, 64], F32R, tag="exir")
        nc.vector.tensor_copy(exir[:], exi[:])
        pbb = p4ps.tile([128, 64], F32, tag="pbb")
        nc.tensor.matmul(pbb[:], r32(ones_r[:]), r32(exir[:]), start=True,
                         stop=True)
        exib = p4.tile([128, 64], F32, tag="exib")
        nc.vector.tensor_copy(exib[:], pbb[:])
        # local rank within chunk
        psl = p4ps.tile([128, 32], F32, tag="psl")
        nc.tensor.matmul(psl[:], r32(tri_sb[:]), r32(g01[:]), start=True,
                         stop=True)
        slf1 = p4.tile([128, 32], F32, tag="slf1")
        nc.vector.tensor_copy(slf1[:], psl[:])
        # one-hot of local rank, masked by routed flag
        oh = p4.tile([128, 32 * 128], F32R, tag="oh")
        oh3 = oh[:].rearrange("p (f r) -> p f r", r=128)
        nc.vector.tensor_tensor(
            out=oh3,
            in0=slf1[:].rearrange("p (f o) -> p f o", o=1).to_broadcast(
                [128, 32, 128]),
            in1=io128[:].rearrange("p (o r) -> p o r", o=1).to_broadcast(
                [128, 32, 128]),
            op=ALU.is_equal)
        nc.vector.tensor_tensor(
            out=oh3, in0=oh3,
            in1=g01[:].rearrange("p (f o) -> p f o", o=1).to_broadcast(
                [128, 32, 128]),
            op=ALU.mult)
        # rhs per chunk: [pid, 1, w, pad]
        rhsA = p4.tile([128, 32 * 4], F32R, tag="rhsA")
        rhs3 = rhsA[:].rearrange("p (f c) -> p f c", c=4)
        nc.vector.tensor_copy(
            rhs3[:, :, 0:1],
            iop[:].rearrange("p (o c) -> p o c", o=1).to_broadcast(
                [128, 32, 1]))
        nc.vector.tensor_copy(
            rhs3[:, :, 1:2],
            ones_cf[:].rearrange("p (o c) -> p o c", o=1).to_broadcast(
                [128, 32, 1]))
        nc.vector.tensor_copy(rhs3[:, :, 2:3],
                              wcol[:].rearrange("p (f c) -> p f c", c=1))
        nc.vector.tensor_copy(
            rhs3[:, :, 3:4],
            ones_cf[:].rearrange("p (o c) -> p o c", o=1).to_broadcast(
                [128, 32, 1]))
        pidx = p4ps.tile([128, 128], F32, tag="pidx")
        for f in range(32):
            nc.tensor.matmul(pidx[:, 4 * f:4 * f + 4],
                             oh[:, f * 128:(f + 1) * 128],
                             rhsA[:, f * 4:(f + 1) * 4],
                             start=True, stop=True)
        idxc2 = p4.tile([128, 128], F32, tag="idxc2")
        nc.vector.tensor_copy(idxc2[:], pidx[:])
        v3 = idxc2[:].rearrange("p (f c) -> p f c", c=4)
        # idx = pid + filled*(128f - N) + N  (sentinel N when empty)
        t1 = p4.tile([128, 32], F32, tag="t1")
        nc.vector.tensor_scalar(out=t1[:], in0=io32[:], scalar1=128.0,
                                scalar2=float(-N), op0=ALU.mult, op1=ALU.add)
        nc.vector.tensor_tensor(
            out=t1[:], in0=t1[:],
            in1=v3[:, :, 1:2].rearrange("p f c -> p (f c)"), op=ALU.mult)
        idxf = p4.tile([128, 32], F32, tag="idxf")
        nc.vector.tensor_tensor(
            out=idxf[:], in0=t1[:],
            in1=v3[:, :, 0:1].rearrange("p f c -> p (f c)"), op=ALU.add)
        nc.vector.tensor_scalar_add(idxf[:], idxf[:], float(N))
        # slot -> (chunk, rank) inverse map
        ge3 = p4.tile([128, 8 * 32], F32, tag="ge3")
        nc.vector.tensor_tensor(
            out=ge3[:].rearrange("p (j f) -> p j f", f=32),
            in0=sg_sb[:].rearrange("p (j o) -> p j o", o=1)
            .to_broadcast([128, 8, 32]),
            in1=exib[:, 32:64].rearrange("p (o f) -> p o f", o=1)
            .to_broadcast([128, 8, 32]),
            op=ALU.is_ge)
        fofs = p4.tile([128, 8], F32, tag="fofs")
        nc.vector.reduce_sum(fofs[:],
                             ge3[:].rearrange("p (j f) -> p j f", f=32),
                             axis=mybir.AxisListType.X)
        ohf = p4.tile([128, 8 * 32], F32, tag="ohf")
        nc.vector.tensor_tensor(
            out=ohf[:].rearrange("p (j f) -> p j f", f=32),
            in0=fofs[:].rearrange("p (j o) -> p j o", o=1)
            .to_broadcast([128, 8, 32]),
            in1=io32[:].rearrange("p (o f) -> p o f", o=1)
            .to_broadcast([128, 8, 32]),
            op=ALU.is_equal)
        nc.vector.tensor_tensor(
            out=ohf[:].rearrange("p (j f) -> p j f", f=32),
            in0=ohf[:].rearrange("p (j f) -> p j f", f=32),
            in1=exib[:, 0:32].rearrange("p (o f) -> p o f", o=1)
            .to_broadcast([128, 8, 32]),
            op=ALU.mult)
        exclsel = p4.tile([128, 8], F32, tag="exclsel")
        nc.vector.reduce_sum(exclsel[:],
                             ohf[:].rearrange("p (j f) -> p j f", f=32),
                             axis=mybir.AxisListType.X)
        # src row = (s - excl[f])*32 + f ; empty slots -> sentinel row 4096
        srcf = p4.tile([128, 8], F32, tag="srcf")
        nc.vector.tensor_tensor(out=srcf[:], in0=sg_sb[:], in1=exclsel[:],
                                op=ALU.subtract)
        nc.vector.tensor_scalar_mul(srcf[:], srcf[:], 32.0)
        nc.vector.tensor_add(srcf[:], srcf[:], fofs[:])
        emp = p4.tile([128, 8], F32, tag="emp")
        nc.vector.tensor_scalar(out=emp[:], in0=fofs[:], scalar1=32.0,
                                scalar2=8192.0, op0=ALU.is_equal,
                                op1=ALU.mult)
        nc.vector.tensor_add(srcf[:], srcf[:], emp[:])
        nc.vector.tensor_scalar(out=srcf[:], in0=srcf[:], scalar1=4096.0,
                                scalar2=None, op0=ALU.min)
        src_i = p4.tile([128, 8], I32, tag="src_i")
        nc.vector.tensor_copy(src_i[:], srcf[:])
        # write (idx, w) table to DRAM, gather back in slot order
        idxw_sb = p4.tile([128, 64], I32, tag="idxw_sb")
        iw3 = idxw_sb[:].rearrange("p (f c) -> p f c", c=2)
        iwf = idxw_sb[:].bitcast(F32).rearrange("p (f c) -> p f c", c=2)
        nc.vector.tensor_copy(
            iw3[:, :, 0:1], idxf[:].rearrange("p (f c) -> p f c", c=1))
        nc.vector.tensor_copy(iwf[:, :, 1:2], v3[:, :, 2:3])
        nc.sync.dma_start(
            idxw_d[0:4096, :].rearrange("(r f) c -> r (f c)", f=32),
            idxw_sb[:])
        sent = p4.tile([128, 2], I32, tag="sent")
        nc.vector.memset(sent[:, 0:1], N)
        nc.vector.memset(sent[:, 1:2], 0)
        nc.sync.dma_start(idxw_d[4096:4224, :], sent[:])
        idxg = p4.tile([128, 16], I32, tag="idxg")
        for j in range(8):
            nc.gpsimd.indirect_dma_start(
                out=idxg[:, 2 * j:2 * j + 2], out_offset=None,
                in_=idxw_d[:, :],
                in_offset=bass.IndirectOffsetOnAxis(
                    ap=src_i[:, j:j + 1], axis=0))
        ig3 = idxg[:].rearrange("p (j c) -> p j c", c=2)
        igf = idxg[:].bitcast(F32).rearrange("p (j c) -> p j c", c=2)
        nc.vector.tensor_copy(
            idx8[:].rearrange("p (j c) -> p j c", c=1), ig3[:, :, 0:1])
        nc.vector.tensor_scalar_mul(
            wslot[:].rearrange("p (j c) -> p j c", c=1), igf[:, :, 1:2],
            1.0 / WS)

    # ---------------- Phase 5: expert FFN (fp8 DoubleRow) + scatter ------
    with tc.tile_pool(name="p5g", bufs=2) as p5g, \
         tc.tile_pool(name="p5", bufs=1) as p5:
        p5ps_cm = tc.tile_pool(name="p5psA", bufs=2, space="PSUM")
        p5ps = p5ps_cm.__enter__()
        h2cT = p5.tile([128, 8 * CAP], FP8, tag="h2cT")
        for j in range(8):
            hc = p5g.tile([128, D], BF16, tag="hc")
            nc.gpsimd.indirect_dma_start(
                out=hc[:], out_offset=None, in_=agout_h2[:, :],
                in_offset=bass.IndirectOffsetOnAxis(
                    ap=idx8[:, j:j + 1], axis=0))
            for dt in range(8):
                pst = p5ps.tile([128, 128], BF16, tag="tp5")
                nc.tensor.transpose(pst[:], hc[:, dt * 128:(dt + 1) * 128],
                                    identb[:])
                nc.vector.tensor_copy(
                    h2cT[:, dt * CAP + j * 128:dt * CAP + (j + 1) * 128],
                    pst[:])
        p5ps_cm.__exit__(None, None, None)
        p5ps_cm2 = tc.tile_pool(name="p5psB", bufs=2, space="PSUM")
        p5ps = p5ps_cm2.__enter__()
        at2sb = p5.tile([128, 32 * CAP], FP8, tag="at2sb")
        wfc3 = wfc_sb[:].rearrange("p (dt m) -> p dt m", m=DFF)
        h2c3 = h2cT[:].rearrange("p (dt m) -> p dt m", m=CAP)
        for gfc in range(32):
            ps1 = p5ps.tile([128, CAP], F32, tag="ps1")
            for dtp in range(4):
                for half in range(2):
                    nc.tensor.matmul(
                        ps1[:, half * 512:(half + 1) * 512],
                        wfc3[:, 2 * dtp:2 * dtp + 2,
                             gfc * 128:(gfc + 1) * 128],
                        h2c3[:, 2 * dtp:2 * dtp + 2,
                             half * 512:(half + 1) * 512],
                        start=(dtp == 0), stop=(dtp == 3),
                        perf_mode=DR)
            nc.scalar.activation(at2sb[:, gfc * CAP:(gfc + 1) * CAP],
                                 ps1[:], AF.Gelu, scale=1.0 / WS)
        at3 = at2sb[:].rearrange("p (g m) -> p g m", m=CAP)
        wpj3 = wpj_sb[:].rearrange("p (gp c2 m) -> p gp c2 m", c2=2, m=D)
        for tt in range(8):
            ps2 = p5ps.tile([128, D], F32, tag="ps2")
            for gp in range(16):
                for half in range(2):
                    nc.tensor.matmul(
                        ps2[:, half * 512:(half + 1) * 512],
                        at3[:, 2 * gp:2 * gp + 2, tt * 128:(tt + 1) * 128],
                        wpj3[:, gp, :, half * 512:(half + 1) * 512],
                        start=(gp == 0), stop=(gp == 15),
                        perf_mode=DR)
            sc = p5g.tile([128, D], BF16, tag="sc")
            nc.scalar.activation(sc[:], ps2[:], AF.Copy,
                                 scale=wslot[:, tt:tt + 1])
            nc.gpsimd.indirect_dma_start(
                out=rsin[:, :],
                out_offset=bass.IndirectOffsetOnAxis(
                    ap=idx8[:, tt:tt + 1], axis=0),
                in_=sc[:], in_offset=None)
        p5ps_cm2.__exit__(None, None, None)
    nc.gpsimd.collective_compute(
        "ReduceScatter", ALU.add, replica_groups=RG8,
        ins=[rsin[0:N, :].opt()], outs=[rsout[:, :].opt()])

    # ---------------- Phase 7: final assembly ----------------
    with tc.tile_pool(name="p7", bufs=2) as p7:
        lnb = p7.tile([128, D], F32, tag="lnb")
        nc.sync.dma_start(lnb[:], ln2bc[:, :])
        for tt in range(4):
            rs = p7.tile([128, D], BF16, tag="rs")
            nc.sync.dma_start(rs[:], rsout[tt * 128:(tt + 1) * 128, :])
            idt = p7.tile([128, D], F32, tag="idt")
            nc.vector.tensor_mul(idt[:], h2sb[:, tt * D:(tt + 1) * D], lnb[:])
            nc.scalar.activation(idt[:], idt[:], AF.Copy,
                                 scale=w8[:, tt:tt + 1])
            nc.vector.tensor_add(idt[:], idt[:], rs[:])
            nc.vector.tensor_add(idt[:], idt[:], xmsb[:, tt * D:(tt + 1) * D])
            nc.sync.dma_start(out[tt * 128:(tt + 1) * 128, :], idt[:])
    for pl in (p5w_p, cmp_p, res_p, cst_p, ident_p):
        pl.release()


def _layernorm(nc, pool, xs, d):
    """LN (no weight) on [128, d] AP; returns (xo, nmean=-mu, rstd)."""
    rsum = pool.tile([128, 1], F32, tag="ln_rsum")
    nc.vector.reduce_sum(rsum[:], xs, axis=mybir.AxisListType.X)
    nmean = pool.tile([128, 1], F32, tag="ln_nmean")
    nc.vector.tensor_scalar_mul(nmean[:], rsum[:], -1.0 / d)
    xc = pool.tile([128, d], F32, tag="ln_xc")
    nc.vector.tensor_scalar_add(xc[:], xs, nmean[:])
    sq = pool.tile([128, d], F32, tag="ln_sq")
    ssum = pool.tile([128, 1], F32, tag="ln_ssum")
    nc.scalar.activation(sq[:], xc[:], AF.Square, accum_out=ssum[:])
    std = pool.tile([128, 1], F32, tag="ln_std")
    nc.scalar.activation(std[:], ssum[:], AF.Sqrt, bias=nc.eps_sb[:],
                         scale=1.0 / d)
    rstd = pool.tile([128, 1], F32, tag="ln_rstd")
    nc.vector.reciprocal(rstd[:], std[:])
    xo = pool.tile([128, d], F32, tag="ln_xo")
    nc.scalar.activation(xo[:], xc[:], AF.Copy, scale=rstd[:])
    return xo, nmean, rstd


# ---------------------------------------------------------------------------
# host side
# ---------------------------------------------------------------------------

def _tile8(a):
    """[1024, m] -> [128, 8*m] with [p, dt*m + j] = a[dt*128 + p, j]."""
    m = a.shape[1]
    return np.ascontiguousarray(
        a.reshape(8, 128, m).transpose(1, 0, 2).reshape(128, 8 * m))


def _host_prep(inputs):
    x = np.asarray(inputs["x"], np.float32).reshape(N, D)
    ln1 = np.asarray(inputs["ln1_w"], np.float64)
    ln2 = np.asarray(inputs["ln2_w"], np.float64)
    wqkv64 = np.asarray(inputs["Wqkv"], np.float64) * ln1[:, None]
    wproj64 = np.asarray(inputs["Wproj"], np.float64)
    wrouter64 = np.asarray(inputs["router_W"], np.float64) * ln2[:, None]
    wqkv = wqkv64.astype(np.float32)
    wqkv[:, 0:D] *= 0.125          # fold 1/sqrt(HD) into q
    wproj = wproj64.astype(np.float32)
    wrouter = wrouter64.astype(np.float32)
    srow = wrouter64.sum(0).astype(np.float32).reshape(1, E + 1)
    # Wvp[d, h*9+j] = sum_m Wv[d, 64h+m] * Pr[64h+m, j],  Pr = Wproj@W~r
    Pr = wproj64 @ wrouter64
    wvp = np.zeros((D, H * 9), np.float64)
    for h in range(H):
        wvp[:, h * 9:(h + 1) * 9] = (
            wqkv64[:, 2 * D + h * HD:2 * D + (h + 1) * HD]
            @ Pr[h * HD:(h + 1) * HD, :])
    wvp = wvp.astype(np.float32)
    wfc64 = np.asarray(inputs["W_fc"], np.float64) * ln2[None, :, None]
    wpj64 = np.asarray(inputs["W_pj"], np.float64)
    wfc8 = np.clip(wfc64 * WS, -240, 240).astype(ml_dtypes.float8_e4m3fn)
    wpj8 = np.clip(wpj64 * WS, -240, 240).astype(ml_dtypes.float8_e4m3fn)
    ln2bc = np.broadcast_to(ln2.astype(np.float32), (128, D)).copy()
    tri = (np.arange(128)[:, None] < np.arange(128)[None, :]).astype(
        np.float32)
    t32e = (np.arange(32)[:, None] < np.arange(32)[None, :])
    t32i = (np.arange(32)[:, None] <= np.arange(32)[None, :])
    tri32x = np.concatenate([t32e, t32i], axis=1).astype(np.float32)
    sgrid = (np.arange(128)[:, None] + 128 * np.arange(8)[None, :]).astype(
        np.float32)
    iota32 = np.broadcast_to(
        np.arange(32, dtype=np.float32)[None, :], (128, 32)).copy()
    iotap = np.arange(128, dtype=np.float32)[:, None].copy()
    iota128r = np.broadcast_to(
        np.arange(128, dtype=np.float32)[None, :], (128, 128)).copy()

    wvp_t = _tile8(wvp)
    wr_t = _tile8(wrouter)

    p = np.arange(128)[:, None]
    q = np.arange(OWN)[None, :]
    ch = np.arange(NCH)

    in_maps = []
    for c in range(NC):
        x_c = x[OWN * c:OWN * (c + 1)]
        bi = c % 4
        # dmask16[p, ch*512 + q] = (512*bi + q >= 128*ch + p)
        dmm = np.zeros((128, NCH, OWN), np.float32)
        for cc in range(NCH):
            dmm[:, cc, :] = (512 * bi + q >= 128 * cc + p)
        dmask = dmm.reshape(128, NCH * OWN).astype(ml_dtypes.bfloat16)
        em = np.zeros((128, 16), np.float32)
        em[:, c] = 1.0
        wfc8_t = np.ascontiguousarray(
            wfc8[c].reshape(8, 128, DFF).transpose(1, 0, 2)
            .reshape(128, 8 * DFF))
        wpj8_t = np.ascontiguousarray(
            wpj8[c].reshape(16, 2, 128, D).transpose(2, 0, 1, 3)
            .reshape(128, 32 * D))
        in_maps.append({
            "x_own": np.ascontiguousarray(x_c),
            "wqkv": wqkv, "wvp_t": wvp_t, "wproj": wproj, "wr_t": wr_t,
            "srow": srow,
            "wfc8_t": wfc8_t, "wpj8_t": wpj8_t,
            "ln2bc": ln2bc, "tri": tri, "tri32x": tri32x,
            "dmask16": dmask, "emask": em, "sgrid": sgrid,
            "iota32": iota32, "iotap": iotap, "iota128r": iota128r,
        })
    return in_maps


def _host_assemble(results):
    return np.concatenate(
        [results[c]["out"] for c in range(NC)], axis=0).reshape(B, T, D)


_NC_CACHE = None


def _get_nc():
    global _NC_CACHE
    if _NC_CACHE is None:
        _NC_CACHE = build_nc()
    return _NC_CACHE


def kernel(**inputs):
    from concourse import bass_utils
    nc = _get_nc()
    in_maps = _host_prep(inputs)
    res = bass_utils.run_bass_kernel_spmd(nc, in_maps,
                                          core_ids=list(range(NC)))
    return _host_assemble(res.results)


if __name__ == "__main__":
    nc = build_nc()
    print("built ok")
